# revision 59
# baseline (speedup 1.0000x reference)
"""GPT-2 (L=12, E=1024, H=16, T=1024, B=8) forward on 8 Trainium2 NeuronCores.

Strategy: data-parallel over batch (1 sequence per core) for the 12 transformer
layers, then a device-side AllGather of the 8 last-position vectors and a
vocab-parallel lm_head (each core computes a V/8 logits shard for all 8
sequences) fused into the same NEFF.

Host-side preprocessing (all linear folds, no model compute):
  - embedding gather x0 = wte[idx] + wpe  (pure indexing)
  - layernorm scale folded into the following matmul weights
  - 1/sqrt(D) folded into W_q
  - wte transposed (+ lnf scale) for the lm_head
  - all layer weights cast to bf16 and repacked into contiguous per-tile
    blocks so every weight DMA is a sequential read

Device layouts per core (P = 128 partitions):
  - residual X: [t=128, tt=8, e=1024] f32 (token-major)
  - LN output transposed HT: [e=128, eo=8, t=1024] bf16 via PE transposes
    (stats for all 8 token tiles first, then h + transposes stream)
  - q,k transposed [c, t] bf16; v natural [t, c] bf16 with a ones column
    appended per head (65-wide lhsT) so the AV matmul also produces the
    softmax denominator as psum row 64 (no separate denominator matmuls)
  - attention transposed: attT[k, q] = (kT-slice).T @ qT into [128,1024]
    2-bank psums; additive -30 causal mask on the diag block via VectorE;
    exp on ScalarE psum->sbuf (bf16); per-q normalization at AV-output
    evacuation via reciprocal_approx_fast (SBUF-staged) + broadcast DMA
  - QK matmuls for head pairs are row-tiled (K=64, rows 0-63 / 64-127) and
    emitted adjacently per j-block so both run concurrently on the PE array
  - qkv matmul units for later head pairs are interleaved into the QK j-loop
    so PE stays busy (and HAM stays warm) while ScalarE drains exp
  - denominator DMAs ride the gpsimd queue so they never head-of-line-block
    weight prefetches on the sync queue
"""

import os
import sys

import numpy as np

sys.path.insert(0, "/opt/trn_rl_repo")

V, BLK, L, H, E = 50257, 1024, 12, 16, 1024
T = 1024
D = E // H  # 64
NCORES = 8
E3 = 3 * E
E4 = 4 * E
NTT = T // 128  # 8 token tiles
NEO = E // 128  # 8 embed tiles
VSH = (V + NCORES - 1) // NCORES  # 6283 vocab shard
VSP = 13 * 512  # 6656 padded shard width
NL = int(os.environ.get("GPT_NL", str(L)))
# debug flags for isolating numerics regressions
F_RSQRT = os.environ.get("GPT_RSQRT", "fused")  # fused | split
F_RECIP = os.environ.get("GPT_RECIP", "approx_sbuf")  # exact | approx | approx_sbuf
F_P2 = os.environ.get("GPT_P2", "merged")  # split | merged

_CACHE = {}


def _build_phase1(nl):
    import concourse.mybir as mybir
    import concourse.tile as tile
    from concourse import bacc
    from concourse.masks import make_identity

    f32 = mybir.dt.float32
    bf16 = mybir.dt.bfloat16
    AF = mybir.ActivationFunctionType
    ALU = mybir.AluOpType

    nc = bacc.Bacc("TRN2", target_bir_lowering=False, num_devices=NCORES)

    x0 = nc.dram_tensor("x0", [T, E], f32, kind="ExternalInput")
    # weights repacked host-side into contiguous per-tile layouts
    wqk = nc.dram_tensor("wqk", [nl, 16, NEO, 128, 128], bf16, kind="ExternalInput")
    wv = nc.dram_tensor("wv", [nl, 2, NEO, 128, 512], bf16, kind="ExternalInput")
    wproj = nc.dram_tensor("wproj", [nl, 2, NEO, 128, 512], bf16, kind="ExternalInput")
    wfc = nc.dram_tensor("wfc", [nl, 32, NEO, 128, 128], bf16, kind="ExternalInput")
    wfc2 = nc.dram_tensor("wfc2", [nl, 4, 2, 8, 128, 512], bf16, kind="ExternalInput")
    xlast = nc.dram_tensor("xlast", [1, E], f32, kind="ExternalOutput")
    if F_P2 == "merged":
        wtet = nc.dram_tensor("wtet", [E, VSP], bf16, kind="ExternalInput")
        lg = nc.dram_tensor("lg", [NCORES, VSP], f32, kind="ExternalOutput")
        cc_in = nc.dram_tensor("cc_in", [1, E], f32, kind="Internal")
        cc_out = nc.dram_tensor(
            "cc_out", [NCORES, E], f32, kind="Internal", addr_space="Shared"
        )

    with tile.TileContext(nc) as tc:
        import contextlib

        ctx = contextlib.ExitStack()
        with ctx:
            singles = ctx.enter_context(tc.tile_pool(name="singles", bufs=1))
            wl = ctx.enter_context(tc.tile_pool(name="wl", bufs=16))  # lhsT [128,128]
            wr = ctx.enter_context(tc.tile_pool(name="wr", bufs=10))  # rhs [128,512]
            hpool = ctx.enter_context(tc.tile_pool(name="hpool", bufs=2))
            stat = ctx.enter_context(tc.tile_pool(name="stat", bufs=2))
            bc = ctx.enter_context(tc.tile_pool(name="bc", bufs=2))
            dram = ctx.enter_context(tc.tile_pool(name="dram", bufs=2, space="DRAM"))
            ps = ctx.enter_context(tc.tile_pool(name="ps", bufs=2, space="PSUM"))
            psb = ctx.enter_context(tc.tile_pool(name="psb", bufs=2, space="PSUM"))

            # ---- persistent tiles ----
            X = singles.tile([128, NTT, T], f32)  # residual [t, tt, e]
            HT = singles.tile([128, NEO, T], bf16)  # ln-out transposed [e, eo, t]
            AOT = singles.tile([128, NEO, T], bf16)  # attn outT [c, co, t]
            # time-shared scratch: attn QT/KT vs mlp H1T; attn V+attT vs mlp FC2A
            scrA = ctx.enter_context(tc.tile_pool(name="scrA", bufs=1))
            scrB = ctx.enter_context(tc.tile_pool(name="scrB", bufs=1))

            ident = singles.tile([128, 128], bf16)
            make_identity(nc, ident)
            eps_t = singles.tile([128, 1], f32)
            nc.gpsimd.memset(eps_t, 1e-5)
            # maskT[k, q] = 0 if q >= k else -30 (additive transposed causal
            # mask); added to a QK psum diag block via ident.T @ maskT = maskT
            maskT = singles.tile([128, 128], bf16)
            nc.gpsimd.memset(maskT, 0.0)
            nc.gpsimd.affine_select(
                out=maskT,
                in_=maskT,
                compare_op=ALU.is_ge,
                fill=-30.0,
                base=0,
                pattern=[[1, 128]],
                channel_multiplier=-1,
            )

            # ---- load x0 ----
            x0v = x0[:, :].rearrange("(tt p) e -> p tt e", p=128)
            for tt in range(NTT):
                nc.sync.dma_start(X[:, tt, :], x0v[:, tt, :])

            def layernorm_into_HT(src):
                """LN(src[t,tt,e]) -> HT[e,eo,t] (transposed, bf16) via PE.
                Stats for all tiles first, then h + transposes stream so PE
                isn't gated tile-by-tile on the DVE stats chain."""
                mvs = []
                for tt in range(NTT):
                    st = stat.tile([128, 2, 6], f32, tag="bnst", name=f"st{tt}")
                    for c in range(2):
                        nc.vector.bn_stats(st[:, c, :], src[:, tt, c * 512 : (c + 1) * 512])
                    mv = stat.tile([128, 2], f32, tag="bnmv8", name=f"mv{tt}", bufs=NTT)
                    nc.vector.bn_aggr(mv, st)
                    rstd = stat.tile([128, 1], f32, tag="rstd8", name=f"rs{tt}", bufs=NTT)
                    if F_RSQRT == "fused":
                        nc.scalar.activation(rstd, mv[:, 1:2], AF.Abs_reciprocal_sqrt, bias=eps_t)
                    else:
                        nc.scalar.activation(rstd, mv[:, 1:2], AF.Sqrt, bias=eps_t)
                        nc.vector.reciprocal(rstd, rstd)
                    mvs.append((mv, rstd))
                for tt in range(NTT):
                    mv, rstd = mvs[tt]
                    h = hpool.tile([128, T], bf16, tag="h", name=f"h{tt}")
                    nc.vector.tensor_scalar(
                        out=h,
                        in0=src[:, tt, :],
                        scalar1=mv[:, 0:1],
                        scalar2=rstd,
                        op0=ALU.subtract,
                        op1=ALU.mult,
                    )
                    for half in range(2):
                        ptr = ps.tile([128, 4, 128], bf16, tag="mm", name=f"ptr{tt}_{half}")
                        for eq in range(4):
                            eo = half * 4 + eq
                            nc.tensor.transpose(ptr[:, eq, :], h[:, eo * 128 : (eo + 1) * 128], ident)
                        nc.scalar.activation(
                            HT[:, half * 4 : half * 4 + 4, tt * 128 : (tt + 1) * 128],
                            ptr,
                            AF.Copy,
                        )

            def mm_lhsw_to_ct(dst, wtiles, cts, act=None):
                """dst[:, i, t] = wtiles[ct].T @ HT for i, ct in enumerate(cts).
                wtiles[ct][eo] is a contiguous [128,128] DRAM tile.
                act=None -> DVE copy evac; else ScalarE activation evac."""
                for i, ct in enumerate(cts):
                    wts = []
                    for eo in range(NEO):
                        wt = wl.tile([128, 128], bf16, tag="w_l", name=f"wl{ct}_{eo}")
                        nc.sync.dma_start(wt, wtiles[ct, eo])
                        wts.append(wt)
                    # both 512-chunks per eo so consecutive matmuls share the
                    # stationary operand (halves LDWEIGHTS traffic)
                    pts = [
                        ps.tile([128, 512], f32, tag="mm", name=f"p{ct}_{ch}")
                        for ch in range(2)
                    ]
                    for eo in range(NEO):
                        for ch in range(2):
                            nc.tensor.matmul(
                                pts[ch],
                                wts[eo],
                                HT[:, eo, ch * 512 : (ch + 1) * 512],
                                start=(eo == 0),
                                stop=(eo == NEO - 1),
                                skip_group_check=True,
                            )
                    for ch in range(2):
                        d = dst[:, i, ch * 512 : (ch + 1) * 512]
                        if act is None:
                            nc.vector.tensor_copy(d, pts[ch])
                        else:
                            nc.scalar.activation(d, pts[ch], act)

            def mm_rhs_phase(lhsT3, rtiles, out_fn, n_k, tts=range(NTT)):
                """out[tt] = sum_k lhsT3[:, k, tt*128:+128].T @ rtiles[k].
                rtiles[k] is a contiguous [128,512] DRAM tile."""
                wts = []
                for k in range(n_k):
                    wt = wr.tile([128, 512], bf16, tag="w_r", name=f"wr{k}")
                    nc.sync.dma_start(wt, rtiles[k])
                    wts.append(wt)
                for tt in tts:
                    pt = ps.tile([128, 512], f32, tag="mm", name=f"pv{tt}")
                    for k in range(n_k):
                        nc.tensor.matmul(
                            pt,
                            lhsT3[:, k, tt * 128 : (tt + 1) * 128],
                            wts[k],
                            start=(k == 0),
                            stop=(k == n_k - 1),
                        )
                    out_fn(tt, pt)

            for l in range(nl):
                # ===== attention =====
                layernorm_into_HT(X)
                qk = scrA.tile([128, 16, T], bf16, tag="scrA", name=f"qk{l}")
                QT = qk[:, 0:8, :]
                KT = qk[:, 8:16, :]
                vatt = scrB.tile([128, 26752], bf16, tag="scrB", name=f"vatt{l}")
                VG = vatt[:, 0:8320].rearrange("p (tt h c) -> p tt h c", tt=NTT, h=H)
                attbuf = [vatt[:, 8320 + i * 4608 : 8320 + (i + 1) * 4608] for i in range(4)]
                # ones column per head for the fused softmax denominator
                nc.gpsimd.memset(vatt[:, 0:8320].rearrange("p (a c) -> p a c", c=65)[:, :, 64:65], 1.0)

                def emit_q(ct):
                    mm_lhsw_to_ct(QT[:, ct : ct + 1, :], wqk[l], [ct])

                def emit_k(ct):
                    mm_lhsw_to_ct(KT[:, ct : ct + 1, :], wqk[l], [8 + ct])

                def v_out(tt, pt, ch):
                    # psum [128, 512] = heads ch*8..ch*8+7, 64 cols each -> 65-strided VG
                    nc.vector.tensor_copy(
                        VG[:, tt, ch * 8 : (ch + 1) * 8, 0:64],
                        pt.rearrange("p (h c) -> p h c", h=8),
                    )

                def emit_v(ch, tts):
                    mm_rhs_phase(
                        HT,
                        wv[l, ch],
                        lambda tt, pt, ch=ch: v_out(tt, pt, ch),
                        NEO,
                        tts=tts,
                    )

                # upfront: Q/K for heads 0-3, V for heads 0-7
                emit_q(0)
                emit_k(0)
                emit_q(1)
                emit_k(1)
                emit_v(0, range(NTT))

                # per-head-slot extra qkv units (unit before head 2p needs ct p done)
                units = {
                    0: [lambda: emit_q(2)],
                    1: [lambda: emit_k(2)],
                    2: [lambda: emit_q(3)],
                    3: [lambda: emit_k(3)],
                    4: [lambda: emit_q(4), lambda: emit_k(4)],
                    5: [lambda: emit_v(1, range(0, 4))],
                    6: [lambda: emit_v(1, range(4, NTT))],
                    7: [lambda: emit_q(5), lambda: emit_k(5)],
                    9: [lambda: emit_q(6), lambda: emit_k(6)],
                    11: [lambda: emit_q(7), lambda: emit_k(7)],
                }

                def head_qk_pair(h0, h1, at0, at1, fillers=()):
                    """QK^T + exp + causal mask for a head pair. The two heads'
                    matmuls are K=64 row-tiled (rows 0-63 / 64-127) and emitted
                    adjacently per j so they run concurrently on the PE array.
                    fillers: closures emitting independent PE work, interleaved
                    into the j loop so PE stays busy while ScalarE drains exp."""
                    fillers = list(fillers)
                    offs = []
                    col = 0
                    for j in range(NTT):
                        qn = T - j * 128
                        offs.append(col)
                        col += qn
                    for j in range(NTT):
                        if j in (2, 4, 6) and fillers:
                            fillers.pop(0)()
                        qn = T - j * 128
                        c0 = offs[j]
                        for hh, attT in ((h0, at0), (h1, at1)):
                            ct, ro = hh // 2, (hh % 2) * 64
                            qT = QT[ro : ro + 64, ct, :]
                            kT = KT[ro : ro + 64, ct, :]
                            pa = ps.tile([128, 1024], f32, tag="mm", name=f"pa{l}_{hh}_{j}")
                            for ch in range(0, qn, 512):
                                w = min(512, qn - ch)
                                nc.tensor.matmul(
                                    pa[:, ch : ch + w],
                                    kT[:, j * 128 : (j + 1) * 128],
                                    qT[:, j * 128 + ch : j * 128 + ch + w],
                                    start=True,
                                    stop=True,
                                    skip_group_check=True,
                                )
                            # additive causal mask on the diagonal 128 cols
                            # (VectorE has slack; keeps PE free for matmuls)
                            nc.vector.tensor_tensor(
                                pa[:, 0:128], pa[:, 0:128], maskT, ALU.add
                            )
                            nc.scalar.activation(attT[:, c0 : c0 + qn], pa[:, :qn], AF.Exp)
                    for fn in fillers:  # drain any leftovers
                        fn()
                    return offs

                def head_av(hh, attT, offs):
                    """AV (with fused denominator row) + normalize into AOT."""
                    av_ps = psb.tile([65, 1024], f32, tag="av", name=f"av{l}_{hh}")
                    for j in range(NTT):
                        for ca in range(2):
                            lo = ca * 512
                            if lo + 512 <= j * 128:
                                continue
                            s = max(lo, j * 128)
                            w = lo + 512 - s
                            nc.tensor.matmul(
                                av_ps[:, s : s + w],
                                VG[:, j, hh, 0:65],
                                attT[:, offs[j] + (s - j * 128) : offs[j] + (s - j * 128) + w],
                                start=(j == 0),
                                stop=(j == 4 * ca + 3),
                                skip_group_check=True,
                            )
                    rden = stat.tile([1, 1024], f32, tag="rden", name=f"rd{l}_{hh}")
                    if F_RECIP == "approx":
                        nc.vector.reciprocal_approx_fast(rden, av_ps[64:65, :])
                    elif F_RECIP == "approx_sbuf":
                        dsb = stat.tile([1, 1024], f32, tag="dsb", name=f"ds{l}_{hh}")
                        nc.scalar.activation(dsb, av_ps[64:65, :], AF.Copy)
                        nc.vector.reciprocal_approx_fast(rden, dsb)
                    else:
                        nc.vector.reciprocal(rden, av_ps[64:65, :])
                    rdd = dram.tile([1, 1024], f32, tag="rdd", name=f"rdd{l}_{hh}")
                    # gpsimd queue: keeps the sem-waiting denominator DMAs from
                    # head-of-line-blocking weight prefetches on the sync queue
                    nc.gpsimd.dma_start(rdd, rden)
                    rdb = bc.tile([64, 1024], f32, tag="rdb", name=f"rdb{l}_{hh}")
                    nc.gpsimd.dma_start(rdb, rdd.to_broadcast([64, 1024]))
                    co, ro2 = hh // 2, (hh % 2) * 64
                    nc.vector.tensor_tensor(
                        AOT[ro2 : ro2 + 64, co, :], av_ps[0:64, :], rdb, ALU.mult
                    )

                # head loop: pairs share the PE array via row tiling (K=64)
                for p in range(8):
                    h0, h1 = 2 * p, 2 * p + 1
                    at0 = attbuf[(2 * p) % 4]
                    at1 = attbuf[(2 * p + 1) % 4]
                    # qkv units interleaved into the QK j loop keep PE busy
                    # while ScalarE drains exp
                    offs = head_qk_pair(
                        h0, h1, at0, at1,
                        fillers=units.get(h0, []) + units.get(h1, []),
                    )
                    head_av(h0, at0, offs)
                    head_av(h1, at1, offs)

                # proj + residual
                for ch in range(2):

                    def proj_out(tt, pt, ch=ch):
                        nc.vector.tensor_tensor(
                            X[:, tt, ch * 512 : (ch + 1) * 512],
                            X[:, tt, ch * 512 : (ch + 1) * 512],
                            pt,
                            ALU.add,
                        )

                    mm_rhs_phase(AOT, wproj[l, ch], proj_out, NEO)

                # ===== mlp =====
                layernorm_into_HT(X)
                H1T = scrA.tile([128, 8, T], bf16, tag="scrA", name=f"h1t{l}")
                FC2A = scrB.tile([128, NTT, T], f32, tag="scrB", name=f"fc2a{l}")
                for slab in range(4):  # 4E in 4 slabs of 1024
                    mm_lhsw_to_ct(H1T, wfc[l], range(slab * 8, slab * 8 + 8), act=AF.Gelu_apprx_tanh)
                    for ch in range(2):
                        if slab == 0:

                            def fc2_out(tt, pt, ch=ch):
                                nc.vector.tensor_tensor(
                                    FC2A[:, tt, ch * 512 : (ch + 1) * 512],
                                    X[:, tt, ch * 512 : (ch + 1) * 512],
                                    pt,
                                    ALU.add,
                                )
                        elif slab < 3:

                            def fc2_out(tt, pt, ch=ch):
                                nc.vector.tensor_tensor(
                                    FC2A[:, tt, ch * 512 : (ch + 1) * 512],
                                    FC2A[:, tt, ch * 512 : (ch + 1) * 512],
                                    pt,
                                    ALU.add,
                                )
                        else:

                            def fc2_out(tt, pt, ch=ch):
                                nc.vector.tensor_tensor(
                                    X[:, tt, ch * 512 : (ch + 1) * 512],
                                    FC2A[:, tt, ch * 512 : (ch + 1) * 512],
                                    pt,
                                    ALU.add,
                                )

                        mm_rhs_phase(H1T, wfc2[l, slab, ch], fc2_out, 8)

            # ===== final layernorm on last token tile, emit last row =====
            st = stat.tile([128, 2, 6], f32, tag="bnst", name="stf")
            for c in range(2):
                nc.vector.bn_stats(st[:, c, :], X[:, NTT - 1, c * 512 : (c + 1) * 512])
            mv = stat.tile([128, 2], f32, tag="bnmv", name="mvf")
            nc.vector.bn_aggr(mv, st)
            rstd = stat.tile([128, 1], f32, tag="rstd", name="rsf")
            if F_RSQRT == "fused":
                nc.scalar.activation(rstd, mv[:, 1:2], AF.Abs_reciprocal_sqrt, bias=eps_t)
            else:
                nc.scalar.activation(rstd, mv[:, 1:2], AF.Sqrt, bias=eps_t)
                nc.vector.reciprocal(rstd, rstd)
            xn = hpool.tile([128, T], f32, tag="hf", name="xnf")
            nc.vector.tensor_scalar(
                out=xn,
                in0=X[:, NTT - 1, :],
                scalar1=mv[:, 0:1],
                scalar2=rstd,
                op0=ALU.subtract,
                op1=ALU.mult,
            )
            nc.sync.dma_start(xlast[:, :], xn[127:128, :])

            if F_P2 == "merged":
                # all-gather the 8 cores' last-position vectors, then compute
                # this core's vocab shard of the logits for all 8 sequences
                nc.sync.dma_start(cc_in[:, :], xn[127:128, :])
                nc.gpsimd.collective_compute(
                    "AllGather",
                    ALU.bypass,
                    replica_groups=[list(range(NCORES))],
                    ins=[cc_in[:, :]],
                    outs=[cc_out[:, :]],
                )
                xf = singles.tile([128, NEO, NCORES], f32)
                for eo in range(NEO):
                    nc.sync.dma_start(
                        xf[:, eo, :],
                        cc_out[:, eo * 128 : (eo + 1) * 128].rearrange("s p -> p s"),
                    )
                xt = singles.tile([128, NEO, NCORES], bf16)
                nc.vector.tensor_copy(xt, xf)
                for vc in range(VSP // 512):
                    pt2 = ps.tile([NCORES, 512], f32, tag="mm", name=f"lg{vc}")
                    for eo in range(NEO):
                        wt = wr.tile([128, 512], bf16, tag="w_r", name=f"wv{vc}_{eo}")
                        nc.sync.dma_start(
                            wt, wtet[eo * 128 : (eo + 1) * 128, vc * 512 : (vc + 1) * 512]
                        )
                        nc.tensor.matmul(
                            pt2, xt[:, eo, :], wt, start=(eo == 0), stop=(eo == NEO - 1)
                        )
                    ot = stat.tile([NCORES, 512], f32, tag="lgo", name=f"lo{vc}")
                    nc.scalar.activation(ot, pt2, AF.Copy)
                    nc.sync.dma_start(lg[:, vc * 512 : (vc + 1) * 512], ot)

    nc.compile()
    return nc


def _build_phase2():
    import concourse.mybir as mybir
    import concourse.tile as tile
    from concourse import bacc

    f32 = mybir.dt.float32
    bf16 = mybir.dt.bfloat16
    AF = mybir.ActivationFunctionType

    nc = bacc.Bacc("TRN2", target_bir_lowering=False)
    xallt = nc.dram_tensor("xallt", [E, NCORES], bf16, kind="ExternalInput")
    wtet = nc.dram_tensor("wtet", [E, VSP], bf16, kind="ExternalInput")
    lg = nc.dram_tensor("lg", [NCORES, VSP], f32, kind="ExternalOutput")

    with tile.TileContext(nc) as tc:
        with (
            tc.tile_pool(name="s", bufs=1) as s,
            tc.tile_pool(name="w", bufs=6) as w,
            tc.tile_pool(name="o", bufs=4) as o,
            tc.tile_pool(name="p", bufs=4, space="PSUM") as p,
        ):
            xt = s.tile([128, NEO, NCORES], bf16)
            nc.sync.dma_start(xt, xallt[:, :].rearrange("(eo p) s -> p eo s", p=128))
            for vc in range(VSP // 512):
                pt = p.tile([NCORES, 512], f32, tag="p", name=f"p{vc}")
                for eo in range(NEO):
                    wt = w.tile([128, 512], bf16, tag="w", name=f"w{vc}_{eo}")
                    nc.sync.dma_start(
                        wt, wtet[eo * 128 : (eo + 1) * 128, vc * 512 : (vc + 1) * 512]
                    )
                    nc.tensor.matmul(pt, xt[:, eo, :], wt, start=(eo == 0), stop=(eo == NEO - 1))
                ot = o.tile([NCORES, 512], f32, tag="o", name=f"o{vc}")
                nc.scalar.activation(ot, pt, AF.Copy)
                nc.sync.dma_start(lg[:, vc * 512 : (vc + 1) * 512], ot)
    nc.compile()
    return nc


def _host_prep(idx, wte, wpe, ln1_w, ln1_b, attn_w, attn_b, proj_w, proj_b,
               ln2_w, ln2_b, fc_w, fc_b, fc2_w, fc2_b, lnf_w, lnf_b, nl):
    import ml_dtypes

    f = np.float32
    bf = ml_dtypes.bfloat16
    idx = np.asarray(idx)
    wte = np.asarray(wte, f)
    wpe = np.asarray(wpe, f)
    x0_all = wte[idx] + wpe[None, :T]  # [8, T, E]

    attn_w = np.asarray(attn_w, f)
    ln1_w = np.asarray(ln1_w, f)
    fc_w = np.asarray(fc_w, f)
    ln2_w = np.asarray(ln2_w, f)

    # fold ln scale into following weights; fold 1/sqrt(D) into W_q
    wqkv = attn_w * ln1_w[:, :, None]
    wqkv[:, :, :E] *= 1.0 / np.sqrt(D)
    wfc = fc_w * ln2_w[:, :, None]

    # biases: must be zero (true for this model); free-dim bias adds unsupported
    bqkv = np.einsum("le,lec->lc", np.asarray(ln1_b, f), attn_w) + np.asarray(attn_b, f)
    bfc = np.einsum("le,lec->lc", np.asarray(ln2_b, f), fc_w) + np.asarray(fc_b, f)
    for nm, b in [("bqkv", bqkv), ("proj_b", np.asarray(proj_b, f)),
                  ("bfc", bfc), ("fc2_b", np.asarray(fc2_b, f)),
                  ("lnf_b", np.asarray(lnf_b, f))]:
        assert np.abs(b).max() == 0.0, f"nonzero bias {nm} not supported by this kernel"

    wtet = np.ascontiguousarray((wte * np.asarray(lnf_w, f)[None, :]).T)  # [E, V]
    shards = []
    for c in range(NCORES):
        sl = wtet[:, c * VSH : min(V, (c + 1) * VSH)]
        pad = np.zeros((E, VSP), f)
        pad[:, : sl.shape[1]] = sl
        shards.append(pad.astype(bf))

    # repack weights into contiguous per-tile layouts (see _build_phase1)
    wqkv = wqkv[:nl].astype(bf)
    proj = np.asarray(proj_w, f)[:nl].astype(bf)
    wfc = wfc[:nl].astype(bf)
    fc2 = np.asarray(fc2_w, f)[:nl].astype(bf)
    # wqk: [nl, 16, NEO, 128, 128] (ct 0-7 = Q, 8-15 = K)
    wqk_t = np.ascontiguousarray(
        wqkv[:, :, : 2 * E]
        .reshape(nl, NEO, 128, 16, 128)
        .transpose(0, 3, 1, 2, 4)
    )
    # wv: [nl, 2, NEO, 128, 512]
    wv_t = np.ascontiguousarray(
        wqkv[:, :, 2 * E :]
        .reshape(nl, NEO, 128, 2, 512)
        .transpose(0, 3, 1, 2, 4)
    )
    # wproj: [nl, 2, NEO, 128, 512]
    wproj_t = np.ascontiguousarray(
        proj.reshape(nl, NEO, 128, 2, 512).transpose(0, 3, 1, 2, 4)
    )
    # wfc: [nl, 32, NEO, 128, 128]
    wfc_t = np.ascontiguousarray(
        wfc.reshape(nl, NEO, 128, 32, 128).transpose(0, 3, 1, 2, 4)
    )
    # wfc2: [nl, 4, 2, 8, 128, 512]
    wfc2_t = np.ascontiguousarray(
        fc2.reshape(nl, 4, 8, 128, 2, 512).transpose(0, 1, 4, 2, 3, 5)
    )
    return (
        np.ascontiguousarray(x0_all, f),
        wqk_t,
        wv_t,
        wproj_t,
        wfc_t,
        wfc2_t,
        shards,
    )


def kernel(idx, wte, wpe, ln1_w, ln1_b, attn_w, attn_b, proj_w, proj_b,
           ln2_w, ln2_b, fc_w, fc_b, fc2_w, fc2_b, lnf_w, lnf_b):
    from concourse.bass_utils import run_bass_kernel_spmd

    x0_all, wqk, wv, wproj, wfc, wfc2, shards = _host_prep(
        idx, wte, wpe, ln1_w, ln1_b, attn_w, attn_b, proj_w, proj_b,
        ln2_w, ln2_b, fc_w, fc_b, fc2_w, fc2_b, lnf_w, lnf_b, NL)

    if "p1" not in _CACHE:
        _CACHE["p1"] = _build_phase1(NL)
    nc1 = _CACHE["p1"]
    in_maps = [
        {"x0": x0_all[c], "wqk": wqk, "wv": wv, "wproj": wproj, "wfc": wfc, "wfc2": wfc2}
        for c in range(NCORES)
    ]
    if F_P2 == "merged":
        for c in range(NCORES):
            in_maps[c]["wtet"] = shards[c]
    trace = os.environ.get("GPT_TRACE", "0") == "1"
    r1 = run_bass_kernel_spmd(nc1, in_maps, core_ids=list(range(NCORES)), trace=trace)
    _CACHE["r1"] = r1

    logits = np.zeros((NCORES, 1, V), np.float32)
    if F_P2 == "merged":
        for c in range(NCORES):
            w = min(V, (c + 1) * VSH) - c * VSH
            logits[:, 0, c * VSH : c * VSH + w] = r1.results[c]["lg"][:, :w]
        return logits

    import ml_dtypes

    xall = np.stack([r1.results[c]["xlast"][0] for c in range(NCORES)])  # [8, E]
    xallt = np.ascontiguousarray(xall.T).astype(ml_dtypes.bfloat16)  # [E, 8]

    if "p2" not in _CACHE:
        _CACHE["p2"] = _build_phase2()
    nc2 = _CACHE["p2"]
    in_maps2 = [{"xallt": xallt, "wtet": shards[c]} for c in range(NCORES)]
    r2 = run_bass_kernel_spmd(nc2, in_maps2, core_ids=list(range(NCORES)), trace=trace)
    _CACHE["r2"] = r2

    for c in range(NCORES):
        w = min(V, (c + 1) * VSH) - c * VSH
        logits[:, 0, c * VSH : c * VSH + w] = r2.results[c]["lg"][:, :w]
    return logits


# revision 61
# speedup vs baseline: 1.0947x; 1.0947x over previous
"""GPT-2 (L=12, E=1024, H=16, T=1024, B=8) forward on 8 Trainium2 NeuronCores.

Strategy: data-parallel over batch (1 sequence per core) for the 12 transformer
layers; vocab-parallel lm_head (each core computes a V/8 logits shard for all
8 sequences) as a second small NEFF, with the 8 last-position vectors gathered
on host between phases. (GPT_P2=merged fuses the lm_head into the first NEFF
via a device AllGather; measured slower, so off by default.)

Host-side preprocessing (all linear folds, no model compute):
  - embedding gather x0 = wte[idx] + wpe  (pure indexing)
  - layernorm scale folded into the following matmul weights
  - 1/sqrt(D) folded into W_q
  - wte transposed (+ lnf scale) for the lm_head
  - all layer weights cast to bf16 and repacked into contiguous per-tile
    blocks so every weight DMA is a sequential read

Device layouts per core (P = 128 partitions):
  - residual X: [t=128, tt=8, e=1024] f32 (token-major)
  - LN output transposed HT: [e=128, eo=8, t=1024] bf16 via PE transposes
  - q,k transposed [c, t] bf16; v natural [t, c] bf16 with a ones column
    appended per head (65-wide lhsT) so the AV matmul also produces the
    softmax denominator as psum row 64 (no separate denominator matmuls)
  - attention transposed: attT[k, q] = (kT-slice).T @ qT into [128,1024]
    2-bank psums; additive -30 causal mask accumulated onto the diag block
    by a PE matmul (ident.T @ maskT); exp on ScalarE psum->sbuf (bf16);
    per-q normalization at AV-output evacuation via
    reciprocal_approx_fast (SBUF-staged) + broadcast DMA
  - QK matmuls for head pairs are row-tiled (K=64, rows 0-63 / 64-127) and
    emitted adjacently per j-block so both run concurrently on the PE array
  - qkv matmul units for later head pairs are interleaved into the QK j-loop
    so PE stays busy (and HAM stays warm) while ScalarE drains exp
  - denominator DMAs ride the gpsimd queue so they never head-of-line-block
    weight prefetches on the sync queue
"""

import os
import sys

import numpy as np

sys.path.insert(0, "/opt/trn_rl_repo")

V, BLK, L, H, E = 50257, 1024, 12, 16, 1024
T = 1024
D = E // H  # 64
NCORES = 8
E3 = 3 * E
E4 = 4 * E
NTT = T // 128  # 8 token tiles
NEO = E // 128  # 8 embed tiles
VSH = (V + NCORES - 1) // NCORES  # 6283 vocab shard
VSP = 13 * 512  # 6656 padded shard width
NL = int(os.environ.get("GPT_NL", str(L)))
# debug flags for isolating numerics regressions
F_RSQRT = os.environ.get("GPT_RSQRT", "fused")  # fused | split
F_RECIP = os.environ.get("GPT_RECIP", "approx_sbuf")  # exact | approx | approx_sbuf
F_P2 = os.environ.get("GPT_P2", "split")  # split | merged

_CACHE = {}


def _build_phase1(nl):
    import concourse.mybir as mybir
    import concourse.tile as tile
    from concourse import bacc
    from concourse.masks import make_identity

    f32 = mybir.dt.float32
    bf16 = mybir.dt.bfloat16
    AF = mybir.ActivationFunctionType
    ALU = mybir.AluOpType

    nc = bacc.Bacc("TRN2", target_bir_lowering=False, num_devices=NCORES)

    x0 = nc.dram_tensor("x0", [T, E], f32, kind="ExternalInput")
    # weights repacked host-side into contiguous per-tile layouts
    wqk = nc.dram_tensor("wqk", [nl, 16, NEO, 128, 128], bf16, kind="ExternalInput")
    wv = nc.dram_tensor("wv", [nl, 2, NEO, 128, 512], bf16, kind="ExternalInput")
    wproj = nc.dram_tensor("wproj", [nl, 2, NEO, 128, 512], bf16, kind="ExternalInput")
    wfc = nc.dram_tensor("wfc", [nl, 32, NEO, 128, 128], bf16, kind="ExternalInput")
    wfc2 = nc.dram_tensor("wfc2", [nl, 4, 2, 8, 128, 512], bf16, kind="ExternalInput")
    xlast = nc.dram_tensor("xlast", [1, E], f32, kind="ExternalOutput")
    if F_P2 == "merged":
        wtet = nc.dram_tensor("wtet", [E, VSP], bf16, kind="ExternalInput")
        lg = nc.dram_tensor("lg", [NCORES, VSP], f32, kind="ExternalOutput")
        cc_in = nc.dram_tensor("cc_in", [1, E], f32, kind="Internal")
        cc_out = nc.dram_tensor(
            "cc_out", [NCORES, E], f32, kind="Internal", addr_space="Shared"
        )

    with tile.TileContext(nc) as tc:
        import contextlib

        ctx = contextlib.ExitStack()
        with ctx:
            singles = ctx.enter_context(tc.tile_pool(name="singles", bufs=1))
            wl = ctx.enter_context(tc.tile_pool(name="wl", bufs=16))  # lhsT [128,128]
            wr = ctx.enter_context(tc.tile_pool(name="wr", bufs=12))  # rhs [128,512]
            hpool = ctx.enter_context(tc.tile_pool(name="hpool", bufs=2))
            stat = ctx.enter_context(tc.tile_pool(name="stat", bufs=2))
            bc = ctx.enter_context(tc.tile_pool(name="bc", bufs=2))
            dram = ctx.enter_context(tc.tile_pool(name="dram", bufs=2, space="DRAM"))
            ps = ctx.enter_context(tc.tile_pool(name="ps", bufs=2, space="PSUM"))
            psb = ctx.enter_context(tc.tile_pool(name="psb", bufs=2, space="PSUM"))

            # ---- persistent tiles ----
            X = singles.tile([128, NTT, T], f32)  # residual [t, tt, e]
            HT = singles.tile([128, NEO, T], bf16)  # ln-out transposed [e, eo, t]
            AOT = singles.tile([128, NEO, T], bf16)  # attn outT [c, co, t]
            # time-shared scratch: attn QT/KT vs mlp H1T; attn V+attT vs mlp FC2A
            scrA = ctx.enter_context(tc.tile_pool(name="scrA", bufs=1))
            scrB = ctx.enter_context(tc.tile_pool(name="scrB", bufs=1))

            ident = singles.tile([128, 128], bf16)
            make_identity(nc, ident)
            eps_t = singles.tile([128, 1], f32)
            nc.gpsimd.memset(eps_t, 1e-5)
            # maskT[k, q] = 0 if q >= k else -30 (additive transposed causal
            # mask); added to a QK psum diag block via ident.T @ maskT = maskT
            maskT = singles.tile([128, 128], bf16)
            nc.gpsimd.memset(maskT, 0.0)
            nc.gpsimd.affine_select(
                out=maskT,
                in_=maskT,
                compare_op=ALU.is_ge,
                fill=-30.0,
                base=0,
                pattern=[[1, 128]],
                channel_multiplier=-1,
            )

            # ---- load x0 ----
            x0v = x0[:, :].rearrange("(tt p) e -> p tt e", p=128)
            for tt in range(NTT):
                nc.sync.dma_start(X[:, tt, :], x0v[:, tt, :])

            def layernorm_into_HT(src):
                """LN(src[t,tt,e]) -> HT[e,eo,t] (transposed, bf16) via PE."""
                for tt in range(NTT):
                    st = stat.tile([128, 2, 6], f32, tag="bnst", name=f"st{tt}")
                    for c in range(2):
                        nc.vector.bn_stats(st[:, c, :], src[:, tt, c * 512 : (c + 1) * 512])
                    mv = stat.tile([128, 2], f32, tag="bnmv", name=f"mv{tt}")
                    nc.vector.bn_aggr(mv, st)
                    rstd = stat.tile([128, 1], f32, tag="rstd", name=f"rs{tt}")
                    if F_RSQRT == "fused":
                        nc.scalar.activation(rstd, mv[:, 1:2], AF.Abs_reciprocal_sqrt, bias=eps_t)
                    else:
                        nc.scalar.activation(rstd, mv[:, 1:2], AF.Sqrt, bias=eps_t)
                        nc.vector.reciprocal(rstd, rstd)
                    h = hpool.tile([128, T], bf16, tag="h", name=f"h{tt}")
                    nc.vector.tensor_scalar(
                        out=h,
                        in0=src[:, tt, :],
                        scalar1=mv[:, 0:1],
                        scalar2=rstd,
                        op0=ALU.subtract,
                        op1=ALU.mult,
                    )
                    for half in range(2):
                        ptr = ps.tile([128, 4, 128], bf16, tag="mm", name=f"ptr{tt}_{half}")
                        for eq in range(4):
                            eo = half * 4 + eq
                            nc.tensor.transpose(ptr[:, eq, :], h[:, eo * 128 : (eo + 1) * 128], ident)
                        nc.scalar.activation(
                            HT[:, half * 4 : half * 4 + 4, tt * 128 : (tt + 1) * 128],
                            ptr,
                            AF.Copy,
                        )

            def mm_lhsw_to_ct(dst, wtiles, cts, act=None):
                """dst[:, i, t] = wtiles[ct].T @ HT for i, ct in enumerate(cts).
                wtiles[ct][eo] is a contiguous [128,128] DRAM tile.
                act=None -> DVE copy evac; else ScalarE activation evac."""
                for i, ct in enumerate(cts):
                    wts = []
                    for eo in range(NEO):
                        wt = wl.tile([128, 128], bf16, tag="w_l", name=f"wl{ct}_{eo}")
                        nc.sync.dma_start(wt, wtiles[ct, eo])
                        wts.append(wt)
                    # both 512-chunks per eo so consecutive matmuls share the
                    # stationary operand (halves LDWEIGHTS traffic)
                    pts = [
                        ps.tile([128, 512], f32, tag="mm", name=f"p{ct}_{ch}")
                        for ch in range(2)
                    ]
                    for eo in range(NEO):
                        for ch in range(2):
                            nc.tensor.matmul(
                                pts[ch],
                                wts[eo],
                                HT[:, eo, ch * 512 : (ch + 1) * 512],
                                start=(eo == 0),
                                stop=(eo == NEO - 1),
                                skip_group_check=True,
                            )
                    for ch in range(2):
                        d = dst[:, i, ch * 512 : (ch + 1) * 512]
                        if act is None:
                            nc.vector.tensor_copy(d, pts[ch])
                        else:
                            nc.scalar.activation(d, pts[ch], act)

            def mm_rhs_phase(lhsT3, rtiles, out_fn, n_k, tts=range(NTT)):
                """out[tt] = sum_k lhsT3[:, k, tt*128:+128].T @ rtiles[k].
                rtiles[k] is a contiguous [128,512] DRAM tile."""
                wts = []
                for k in range(n_k):
                    wt = wr.tile([128, 512], bf16, tag="w_r", name=f"wr{k}")
                    nc.sync.dma_start(wt, rtiles[k])
                    wts.append(wt)
                for tt in tts:
                    pt = ps.tile([128, 512], f32, tag="mm", name=f"pv{tt}")
                    for k in range(n_k):
                        nc.tensor.matmul(
                            pt,
                            lhsT3[:, k, tt * 128 : (tt + 1) * 128],
                            wts[k],
                            start=(k == 0),
                            stop=(k == n_k - 1),
                        )
                    out_fn(tt, pt)

            for l in range(nl):
                # ===== attention =====
                layernorm_into_HT(X)
                qk = scrA.tile([128, 16, T], bf16, tag="scrA", name=f"qk{l}")
                QT = qk[:, 0:8, :]
                KT = qk[:, 8:16, :]
                vatt = scrB.tile([128, 26880], bf16, tag="scrB", name=f"vatt{l}")
                VG = vatt[:, 0:8320].rearrange("p (tt h c) -> p tt h c", tt=NTT, h=H)
                attbuf = [vatt[:, 8320 + i * 4640 : 8320 + (i + 1) * 4640] for i in range(4)]
                # ones column per head for the fused softmax denominator
                nc.gpsimd.memset(vatt[:, 0:8320].rearrange("p (a c) -> p a c", c=65)[:, :, 64:65], 1.0)

                def emit_q(ct):
                    mm_lhsw_to_ct(QT[:, ct : ct + 1, :], wqk[l], [ct])

                def emit_k(ct):
                    mm_lhsw_to_ct(KT[:, ct : ct + 1, :], wqk[l], [8 + ct])

                def v_out(tt, pt, ch):
                    # psum [128, 512] = heads ch*8..ch*8+7, 64 cols each -> 65-strided VG
                    nc.vector.tensor_copy(
                        VG[:, tt, ch * 8 : (ch + 1) * 8, 0:64],
                        pt.rearrange("p (h c) -> p h c", h=8),
                    )

                def emit_v(ch, tts):
                    mm_rhs_phase(
                        HT,
                        wv[l, ch],
                        lambda tt, pt, ch=ch: v_out(tt, pt, ch),
                        NEO,
                        tts=tts,
                    )

                # upfront: Q/K for heads 0-3, V for heads 0-7
                emit_q(0)
                emit_k(0)
                emit_q(1)
                emit_k(1)
                emit_v(0, range(NTT))

                # per-head-slot extra qkv units (unit before head 2p needs ct p done)
                units = {
                    0: [lambda: emit_q(2)],
                    1: [lambda: emit_k(2)],
                    2: [lambda: emit_q(3)],
                    3: [lambda: emit_k(3)],
                    4: [lambda: emit_q(4), lambda: emit_k(4)],
                    5: [lambda: emit_v(1, range(0, 4))],
                    6: [lambda: emit_v(1, range(4, NTT))],
                    7: [lambda: emit_q(5), lambda: emit_k(5)],
                    9: [lambda: emit_q(6), lambda: emit_k(6)],
                    11: [lambda: emit_q(7), lambda: emit_k(7)],
                }

                def head_qk_pair(h0, h1, at0, at1, fillers=()):
                    """QK^T + exp + causal mask for a head pair. The two heads'
                    matmuls are K=64 row-tiled (rows 0-63 / 64-127) and emitted
                    adjacently per j so they run concurrently on the PE array.
                    fillers: closures emitting independent PE work, interleaved
                    into the j loop so PE stays busy while ScalarE drains exp."""
                    fillers = list(fillers)
                    offs = []
                    col = 0
                    for j in range(NTT):
                        qn = T - j * 128
                        offs.append(col)
                        col += qn
                    for j in range(NTT):
                        if j in (2, 4, 6) and fillers:
                            fillers.pop(0)()
                        qn = T - j * 128
                        c0 = offs[j]
                        for hh, attT in ((h0, at0), (h1, at1)):
                            ct, ro = hh // 2, (hh % 2) * 64
                            qT = QT[ro : ro + 64, ct, :]
                            kT = KT[ro : ro + 64, ct, :]
                            pa = ps.tile([128, 1024], f32, tag="mm", name=f"pa{l}_{hh}_{j}")
                            for ch in range(0, qn, 512):
                                w = min(512, qn - ch)
                                nc.tensor.matmul(
                                    pa[:, ch : ch + w],
                                    kT[:, j * 128 : (j + 1) * 128],
                                    qT[:, j * 128 + ch : j * 128 + ch + w],
                                    start=True,
                                    stop=(ch > 0),
                                    skip_group_check=True,
                                )
                            # additive causal mask on the diagonal 128 cols via PE
                            nc.tensor.matmul(
                                pa[:, 0:128],
                                ident,
                                maskT,
                                start=False,
                                stop=True,
                                skip_group_check=True,
                            )
                            nc.scalar.activation(attT[:, c0 : c0 + qn], pa[:, :qn], AF.Exp)
                    for fn in fillers:  # drain any leftovers
                        fn()
                    return offs

                def head_av(hh, attT, offs):
                    """AV (with fused denominator row) + normalize into AOT."""
                    av_ps = psb.tile([65, 1024], f32, tag="av", name=f"av{l}_{hh}")
                    for j in range(NTT):
                        for ca in range(2):
                            lo = ca * 512
                            if lo + 512 <= j * 128:
                                continue
                            s = max(lo, j * 128)
                            w = lo + 512 - s
                            nc.tensor.matmul(
                                av_ps[:, s : s + w],
                                VG[:, j, hh, 0:65],
                                attT[:, offs[j] + (s - j * 128) : offs[j] + (s - j * 128) + w],
                                start=(j == 0),
                                stop=(j == 4 * ca + 3),
                                skip_group_check=True,
                            )
                    rden = stat.tile([1, 1024], f32, tag="rden", name=f"rd{l}_{hh}")
                    if F_RECIP == "approx":
                        nc.vector.reciprocal_approx_fast(rden, av_ps[64:65, :])
                    elif F_RECIP == "approx_sbuf":
                        dsb = stat.tile([1, 1024], f32, tag="dsb", name=f"ds{l}_{hh}")
                        nc.scalar.activation(dsb, av_ps[64:65, :], AF.Copy)
                        nc.vector.reciprocal_approx_fast(rden, dsb)
                    else:
                        nc.vector.reciprocal(rden, av_ps[64:65, :])
                    rdd = dram.tile([1, 1024], f32, tag="rdd", name=f"rdd{l}_{hh}")
                    # gpsimd queue: keeps the sem-waiting denominator DMAs from
                    # head-of-line-blocking weight prefetches on the sync queue
                    nc.gpsimd.dma_start(rdd, rden)
                    rdb = bc.tile([64, 1024], f32, tag="rdb", name=f"rdb{l}_{hh}")
                    nc.gpsimd.dma_start(rdb, rdd.to_broadcast([64, 1024]))
                    co, ro2 = hh // 2, (hh % 2) * 64
                    nc.vector.tensor_tensor(
                        AOT[ro2 : ro2 + 64, co, :], av_ps[0:64, :], rdb, ALU.mult
                    )

                # head loop: pairs share the PE array via row tiling (K=64)
                for p in range(8):
                    h0, h1 = 2 * p, 2 * p + 1
                    at0 = attbuf[(2 * p) % 4]
                    at1 = attbuf[(2 * p + 1) % 4]
                    # qkv units interleaved into the QK j loop keep PE busy
                    # while ScalarE drains exp
                    offs = head_qk_pair(
                        h0, h1, at0, at1,
                        fillers=units.get(h0, []) + units.get(h1, []),
                    )
                    head_av(h0, at0, offs)
                    head_av(h1, at1, offs)

                # proj + residual
                for ch in range(2):

                    def proj_out(tt, pt, ch=ch):
                        nc.vector.tensor_tensor(
                            X[:, tt, ch * 512 : (ch + 1) * 512],
                            X[:, tt, ch * 512 : (ch + 1) * 512],
                            pt,
                            ALU.add,
                        )

                    mm_rhs_phase(AOT, wproj[l, ch], proj_out, NEO)

                # ===== mlp =====
                layernorm_into_HT(X)
                H1T = scrA.tile([128, 8, T], bf16, tag="scrA", name=f"h1t{l}")
                FC2A = scrB.tile([128, NTT, T], f32, tag="scrB", name=f"fc2a{l}")
                for slab in range(4):  # 4E in 4 slabs of 1024
                    mm_lhsw_to_ct(H1T, wfc[l], range(slab * 8, slab * 8 + 8), act=AF.Gelu_apprx_tanh)
                    for ch in range(2):
                        if slab == 0:

                            def fc2_out(tt, pt, ch=ch):
                                nc.vector.tensor_tensor(
                                    FC2A[:, tt, ch * 512 : (ch + 1) * 512],
                                    X[:, tt, ch * 512 : (ch + 1) * 512],
                                    pt,
                                    ALU.add,
                                )
                        elif slab < 3:

                            def fc2_out(tt, pt, ch=ch):
                                nc.vector.tensor_tensor(
                                    FC2A[:, tt, ch * 512 : (ch + 1) * 512],
                                    FC2A[:, tt, ch * 512 : (ch + 1) * 512],
                                    pt,
                                    ALU.add,
                                )
                        else:

                            def fc2_out(tt, pt, ch=ch):
                                nc.vector.tensor_tensor(
                                    X[:, tt, ch * 512 : (ch + 1) * 512],
                                    FC2A[:, tt, ch * 512 : (ch + 1) * 512],
                                    pt,
                                    ALU.add,
                                )

                        mm_rhs_phase(H1T, wfc2[l, slab, ch], fc2_out, 8)

            # ===== final layernorm on last token tile, emit last row =====
            st = stat.tile([128, 2, 6], f32, tag="bnst", name="stf")
            for c in range(2):
                nc.vector.bn_stats(st[:, c, :], X[:, NTT - 1, c * 512 : (c + 1) * 512])
            mv = stat.tile([128, 2], f32, tag="bnmv", name="mvf")
            nc.vector.bn_aggr(mv, st)
            rstd = stat.tile([128, 1], f32, tag="rstd", name="rsf")
            if F_RSQRT == "fused":
                nc.scalar.activation(rstd, mv[:, 1:2], AF.Abs_reciprocal_sqrt, bias=eps_t)
            else:
                nc.scalar.activation(rstd, mv[:, 1:2], AF.Sqrt, bias=eps_t)
                nc.vector.reciprocal(rstd, rstd)
            xn = hpool.tile([128, T], f32, tag="hf", name="xnf")
            nc.vector.tensor_scalar(
                out=xn,
                in0=X[:, NTT - 1, :],
                scalar1=mv[:, 0:1],
                scalar2=rstd,
                op0=ALU.subtract,
                op1=ALU.mult,
            )
            nc.sync.dma_start(xlast[:, :], xn[127:128, :])

            if F_P2 == "merged":
                # all-gather the 8 cores' last-position vectors, then compute
                # this core's vocab shard of the logits for all 8 sequences
                nc.sync.dma_start(cc_in[:, :], xn[127:128, :])
                nc.gpsimd.collective_compute(
                    "AllGather",
                    ALU.bypass,
                    replica_groups=[list(range(NCORES))],
                    ins=[cc_in[:, :]],
                    outs=[cc_out[:, :]],
                )
                xf = singles.tile([128, NEO, NCORES], f32)
                for eo in range(NEO):
                    nc.sync.dma_start(
                        xf[:, eo, :],
                        cc_out[:, eo * 128 : (eo + 1) * 128].rearrange("s p -> p s"),
                    )
                xt = singles.tile([128, NEO, NCORES], bf16)
                nc.vector.tensor_copy(xt, xf)
                for vc in range(VSP // 512):
                    pt2 = ps.tile([NCORES, 512], f32, tag="mm", name=f"lg{vc}")
                    for eo in range(NEO):
                        wt = wr.tile([128, 512], bf16, tag="w_r", name=f"wv{vc}_{eo}")
                        nc.sync.dma_start(
                            wt, wtet[eo * 128 : (eo + 1) * 128, vc * 512 : (vc + 1) * 512]
                        )
                        nc.tensor.matmul(
                            pt2, xt[:, eo, :], wt, start=(eo == 0), stop=(eo == NEO - 1)
                        )
                    ot = stat.tile([NCORES, 512], f32, tag="lgo", name=f"lo{vc}")
                    nc.scalar.activation(ot, pt2, AF.Copy)
                    nc.sync.dma_start(lg[:, vc * 512 : (vc + 1) * 512], ot)

    nc.compile()
    return nc


def _build_phase2():
    import concourse.mybir as mybir
    import concourse.tile as tile
    from concourse import bacc

    f32 = mybir.dt.float32
    bf16 = mybir.dt.bfloat16
    AF = mybir.ActivationFunctionType

    nc = bacc.Bacc("TRN2", target_bir_lowering=False)
    xallt = nc.dram_tensor("xallt", [E, NCORES], bf16, kind="ExternalInput")
    wtet = nc.dram_tensor("wtet", [E, VSP], bf16, kind="ExternalInput")
    lg = nc.dram_tensor("lg", [NCORES, VSP], f32, kind="ExternalOutput")

    with tile.TileContext(nc) as tc:
        with (
            tc.tile_pool(name="s", bufs=1) as s,
            tc.tile_pool(name="w", bufs=6) as w,
            tc.tile_pool(name="o", bufs=4) as o,
            tc.tile_pool(name="p", bufs=4, space="PSUM") as p,
        ):
            xt = s.tile([128, NEO, NCORES], bf16)
            nc.sync.dma_start(xt, xallt[:, :].rearrange("(eo p) s -> p eo s", p=128))
            for vc in range(VSP // 512):
                pt = p.tile([NCORES, 512], f32, tag="p", name=f"p{vc}")
                for eo in range(NEO):
                    wt = w.tile([128, 512], bf16, tag="w", name=f"w{vc}_{eo}")
                    nc.sync.dma_start(
                        wt, wtet[eo * 128 : (eo + 1) * 128, vc * 512 : (vc + 1) * 512]
                    )
                    nc.tensor.matmul(pt, xt[:, eo, :], wt, start=(eo == 0), stop=(eo == NEO - 1))
                ot = o.tile([NCORES, 512], f32, tag="o", name=f"o{vc}")
                nc.scalar.activation(ot, pt, AF.Copy)
                nc.sync.dma_start(lg[:, vc * 512 : (vc + 1) * 512], ot)
    nc.compile()
    return nc


def _host_prep(idx, wte, wpe, ln1_w, ln1_b, attn_w, attn_b, proj_w, proj_b,
               ln2_w, ln2_b, fc_w, fc_b, fc2_w, fc2_b, lnf_w, lnf_b, nl):
    import ml_dtypes

    f = np.float32
    bf = ml_dtypes.bfloat16
    idx = np.asarray(idx)
    wte = np.asarray(wte, f)
    wpe = np.asarray(wpe, f)
    x0_all = wte[idx] + wpe[None, :T]  # [8, T, E]

    attn_w = np.asarray(attn_w, f)
    ln1_w = np.asarray(ln1_w, f)
    fc_w = np.asarray(fc_w, f)
    ln2_w = np.asarray(ln2_w, f)

    # fold ln scale into following weights; fold 1/sqrt(D) into W_q
    wqkv = attn_w * ln1_w[:, :, None]
    wqkv[:, :, :E] *= 1.0 / np.sqrt(D)
    wfc = fc_w * ln2_w[:, :, None]

    # biases: must be zero (true for this model); free-dim bias adds unsupported
    bqkv = np.einsum("le,lec->lc", np.asarray(ln1_b, f), attn_w) + np.asarray(attn_b, f)
    bfc = np.einsum("le,lec->lc", np.asarray(ln2_b, f), fc_w) + np.asarray(fc_b, f)
    for nm, b in [("bqkv", bqkv), ("proj_b", np.asarray(proj_b, f)),
                  ("bfc", bfc), ("fc2_b", np.asarray(fc2_b, f)),
                  ("lnf_b", np.asarray(lnf_b, f))]:
        assert np.abs(b).max() == 0.0, f"nonzero bias {nm} not supported by this kernel"

    wtet = np.ascontiguousarray((wte * np.asarray(lnf_w, f)[None, :]).T)  # [E, V]
    shards = []
    for c in range(NCORES):
        sl = wtet[:, c * VSH : min(V, (c + 1) * VSH)]
        pad = np.zeros((E, VSP), f)
        pad[:, : sl.shape[1]] = sl
        shards.append(pad.astype(bf))

    # repack weights into contiguous per-tile layouts (see _build_phase1)
    wqkv = wqkv[:nl].astype(bf)
    proj = np.asarray(proj_w, f)[:nl].astype(bf)
    wfc = wfc[:nl].astype(bf)
    fc2 = np.asarray(fc2_w, f)[:nl].astype(bf)
    # wqk: [nl, 16, NEO, 128, 128] (ct 0-7 = Q, 8-15 = K)
    wqk_t = np.ascontiguousarray(
        wqkv[:, :, : 2 * E]
        .reshape(nl, NEO, 128, 16, 128)
        .transpose(0, 3, 1, 2, 4)
    )
    # wv: [nl, 2, NEO, 128, 512]
    wv_t = np.ascontiguousarray(
        wqkv[:, :, 2 * E :]
        .reshape(nl, NEO, 128, 2, 512)
        .transpose(0, 3, 1, 2, 4)
    )
    # wproj: [nl, 2, NEO, 128, 512]
    wproj_t = np.ascontiguousarray(
        proj.reshape(nl, NEO, 128, 2, 512).transpose(0, 3, 1, 2, 4)
    )
    # wfc: [nl, 32, NEO, 128, 128]
    wfc_t = np.ascontiguousarray(
        wfc.reshape(nl, NEO, 128, 32, 128).transpose(0, 3, 1, 2, 4)
    )
    # wfc2: [nl, 4, 2, 8, 128, 512]
    wfc2_t = np.ascontiguousarray(
        fc2.reshape(nl, 4, 8, 128, 2, 512).transpose(0, 1, 4, 2, 3, 5)
    )
    return (
        np.ascontiguousarray(x0_all, f),
        wqk_t,
        wv_t,
        wproj_t,
        wfc_t,
        wfc2_t,
        shards,
    )


def kernel(idx, wte, wpe, ln1_w, ln1_b, attn_w, attn_b, proj_w, proj_b,
           ln2_w, ln2_b, fc_w, fc_b, fc2_w, fc2_b, lnf_w, lnf_b):
    from concourse.bass_utils import run_bass_kernel_spmd

    x0_all, wqk, wv, wproj, wfc, wfc2, shards = _host_prep(
        idx, wte, wpe, ln1_w, ln1_b, attn_w, attn_b, proj_w, proj_b,
        ln2_w, ln2_b, fc_w, fc_b, fc2_w, fc2_b, lnf_w, lnf_b, NL)

    if "p1" not in _CACHE:
        _CACHE["p1"] = _build_phase1(NL)
    nc1 = _CACHE["p1"]
    in_maps = [
        {"x0": x0_all[c], "wqk": wqk, "wv": wv, "wproj": wproj, "wfc": wfc, "wfc2": wfc2}
        for c in range(NCORES)
    ]
    if F_P2 == "merged":
        for c in range(NCORES):
            in_maps[c]["wtet"] = shards[c]
    trace = os.environ.get("GPT_TRACE", "0") == "1"
    r1 = run_bass_kernel_spmd(nc1, in_maps, core_ids=list(range(NCORES)), trace=trace)
    _CACHE["r1"] = r1

    logits = np.zeros((NCORES, 1, V), np.float32)
    if F_P2 == "merged":
        for c in range(NCORES):
            w = min(V, (c + 1) * VSH) - c * VSH
            logits[:, 0, c * VSH : c * VSH + w] = r1.results[c]["lg"][:, :w]
        return logits

    import ml_dtypes

    xall = np.stack([r1.results[c]["xlast"][0] for c in range(NCORES)])  # [8, E]
    xallt = np.ascontiguousarray(xall.T).astype(ml_dtypes.bfloat16)  # [E, 8]

    if "p2" not in _CACHE:
        _CACHE["p2"] = _build_phase2()
    nc2 = _CACHE["p2"]
    in_maps2 = [{"xallt": xallt, "wtet": shards[c]} for c in range(NCORES)]
    r2 = run_bass_kernel_spmd(nc2, in_maps2, core_ids=list(range(NCORES)), trace=trace)
    _CACHE["r2"] = r2

    for c in range(NCORES):
        w = min(V, (c + 1) * VSH) - c * VSH
        logits[:, 0, c * VSH : c * VSH + w] = r2.results[c]["lg"][:, :w]
    return logits


# revision 62
# speedup vs baseline: 1.1075x; 1.0117x over previous
"""GPT-2 (L=12, E=1024, H=16, T=1024, B=8) forward on 8 Trainium2 NeuronCores.

Strategy: data-parallel over batch (1 sequence per core) for the 12 transformer
layers; vocab-parallel lm_head (each core computes a V/8 logits shard for all
8 sequences) as a second small NEFF, with the 8 last-position vectors gathered
on host between phases. (GPT_P2=merged fuses the lm_head into the first NEFF
via a device AllGather; measured slower, so off by default.)

Host-side preprocessing (all linear folds, no model compute):
  - embedding gather x0 = wte[idx] + wpe  (pure indexing)
  - layernorm scale folded into the following matmul weights
  - 1/sqrt(D) folded into W_q
  - wte transposed (+ lnf scale) for the lm_head
  - all layer weights cast to bf16 and repacked into contiguous per-tile
    blocks so every weight DMA is a sequential read

Device layouts per core (P = 128 partitions):
  - residual X: [t=128, tt=8, e=1024] f32 (token-major)
  - LN output transposed HT: [e=128, eo=8, t=1024] bf16 via PE transposes
  - q,k transposed [c, t] bf16; v natural [t, c] bf16 with a ones column
    appended per head (65-wide lhsT) so the AV matmul also produces the
    softmax denominator as psum row 64 (no separate denominator matmuls)
  - attention transposed: attT[k, q] = (kT-slice).T @ qT into [128,1024]
    2-bank psums; additive -30 causal mask accumulated onto the diag block
    by a PE matmul (ident.T @ maskT); exp on ScalarE psum->sbuf (bf16);
    per-q normalization at AV-output evacuation via
    reciprocal_approx_fast (SBUF-staged) + broadcast DMA
  - QK matmuls for head pairs are row-tiled (K=64, rows 0-63 / 64-127) and
    emitted adjacently per j-block so both run concurrently on the PE array
  - qkv matmul units for later head pairs are interleaved into the QK j-loop
    so PE stays busy (and HAM stays warm) while ScalarE drains exp
  - denominator DMAs ride the gpsimd queue so they never head-of-line-block
    weight prefetches on the sync queue
"""

import os
import sys

import numpy as np

sys.path.insert(0, "/opt/trn_rl_repo")

V, BLK, L, H, E = 50257, 1024, 12, 16, 1024
T = 1024
D = E // H  # 64
NCORES = 8
E3 = 3 * E
E4 = 4 * E
NTT = T // 128  # 8 token tiles
NEO = E // 128  # 8 embed tiles
VSH = (V + NCORES - 1) // NCORES  # 6283 vocab shard
VSP = 13 * 512  # 6656 padded shard width
NL = int(os.environ.get("GPT_NL", str(L)))
# debug flags for isolating numerics regressions
F_RSQRT = os.environ.get("GPT_RSQRT", "fused")  # fused | split
F_RECIP = os.environ.get("GPT_RECIP", "approx_sbuf")  # exact | approx | approx_sbuf
F_P2 = os.environ.get("GPT_P2", "split")  # split | merged

_CACHE = {}


def _build_phase1(nl):
    import concourse.mybir as mybir
    import concourse.tile as tile
    from concourse import bacc
    from concourse.masks import make_identity

    f32 = mybir.dt.float32
    bf16 = mybir.dt.bfloat16
    AF = mybir.ActivationFunctionType
    ALU = mybir.AluOpType

    nc = bacc.Bacc("TRN2", target_bir_lowering=False, num_devices=NCORES)

    x0 = nc.dram_tensor("x0", [T, E], f32, kind="ExternalInput")
    # weights repacked host-side into contiguous per-tile layouts
    wqk = nc.dram_tensor("wqk", [nl, 16, NEO, 128, 128], bf16, kind="ExternalInput")
    wv = nc.dram_tensor("wv", [nl, 2, NEO, 128, 512], bf16, kind="ExternalInput")
    wproj = nc.dram_tensor("wproj", [nl, 2, NEO, 128, 512], bf16, kind="ExternalInput")
    wfc = nc.dram_tensor("wfc", [nl, 32, NEO, 128, 128], bf16, kind="ExternalInput")
    wfc2 = nc.dram_tensor("wfc2", [nl, 4, 2, 8, 128, 512], bf16, kind="ExternalInput")
    xlast = nc.dram_tensor("xlast", [1, E], f32, kind="ExternalOutput")
    if F_P2 == "merged":
        wtet = nc.dram_tensor("wtet", [E, VSP], bf16, kind="ExternalInput")
        lg = nc.dram_tensor("lg", [NCORES, VSP], f32, kind="ExternalOutput")
        cc_in = nc.dram_tensor("cc_in", [1, E], f32, kind="Internal")
        cc_out = nc.dram_tensor(
            "cc_out", [NCORES, E], f32, kind="Internal", addr_space="Shared"
        )

    with tile.TileContext(nc) as tc:
        import contextlib

        ctx = contextlib.ExitStack()
        with ctx:
            singles = ctx.enter_context(tc.tile_pool(name="singles", bufs=1))
            wl = ctx.enter_context(tc.tile_pool(name="wl", bufs=16))  # lhsT [128,128]
            wr = ctx.enter_context(tc.tile_pool(name="wr", bufs=12))  # rhs [128,512]
            hpool = ctx.enter_context(tc.tile_pool(name="hpool", bufs=3))
            stat = ctx.enter_context(tc.tile_pool(name="stat", bufs=2))
            bc = ctx.enter_context(tc.tile_pool(name="bc", bufs=2))
            dram = ctx.enter_context(tc.tile_pool(name="dram", bufs=2, space="DRAM"))
            ps = ctx.enter_context(tc.tile_pool(name="ps", bufs=2, space="PSUM"))
            psb = ctx.enter_context(tc.tile_pool(name="psb", bufs=2, space="PSUM"))

            # ---- persistent tiles ----
            X = singles.tile([128, NTT, T], f32)  # residual [t, tt, e]
            HT = singles.tile([128, NEO, T], bf16)  # ln-out transposed [e, eo, t]
            AOT = singles.tile([128, NEO, T], bf16)  # attn outT [c, co, t]
            # time-shared scratch: attn QT/KT vs mlp H1T; attn V+attT vs mlp FC2A
            scrA = ctx.enter_context(tc.tile_pool(name="scrA", bufs=1))
            scrB = ctx.enter_context(tc.tile_pool(name="scrB", bufs=1))

            ident = singles.tile([128, 128], bf16)
            make_identity(nc, ident)
            eps_t = singles.tile([128, 1], f32)
            nc.gpsimd.memset(eps_t, 1e-5)
            # maskT[k, q] = 0 if q >= k else -30 (additive transposed causal
            # mask); added to a QK psum diag block via ident.T @ maskT = maskT
            maskT = singles.tile([128, 128], bf16)
            nc.gpsimd.memset(maskT, 0.0)
            nc.gpsimd.affine_select(
                out=maskT,
                in_=maskT,
                compare_op=ALU.is_ge,
                fill=-30.0,
                base=0,
                pattern=[[1, 128]],
                channel_multiplier=-1,
            )

            # ---- load x0 ----
            x0v = x0[:, :].rearrange("(tt p) e -> p tt e", p=128)
            for tt in range(NTT):
                nc.sync.dma_start(X[:, tt, :], x0v[:, tt, :])

            def layernorm_into_HT(src):
                """LN(src[t,tt,e]) -> HT[e,eo,t] (transposed, bf16) via PE."""
                for tt in range(NTT):
                    st = stat.tile([128, 2, 6], f32, tag="bnst", name=f"st{tt}")
                    for c in range(2):
                        nc.vector.bn_stats(st[:, c, :], src[:, tt, c * 512 : (c + 1) * 512])
                    mv = stat.tile([128, 2], f32, tag="bnmv", name=f"mv{tt}")
                    nc.vector.bn_aggr(mv, st)
                    rstd = stat.tile([128, 1], f32, tag="rstd", name=f"rs{tt}")
                    if F_RSQRT == "fused":
                        nc.scalar.activation(rstd, mv[:, 1:2], AF.Abs_reciprocal_sqrt, bias=eps_t)
                    else:
                        nc.scalar.activation(rstd, mv[:, 1:2], AF.Sqrt, bias=eps_t)
                        nc.vector.reciprocal(rstd, rstd)
                    h = hpool.tile([128, T], bf16, tag="h", name=f"h{tt}")
                    nc.vector.tensor_scalar(
                        out=h,
                        in0=src[:, tt, :],
                        scalar1=mv[:, 0:1],
                        scalar2=rstd,
                        op0=ALU.subtract,
                        op1=ALU.mult,
                    )
                    for half in range(2):
                        ptr = ps.tile([128, 4, 128], bf16, tag="mm", name=f"ptr{tt}_{half}")
                        for eq in range(4):
                            eo = half * 4 + eq
                            nc.tensor.transpose(ptr[:, eq, :], h[:, eo * 128 : (eo + 1) * 128], ident)
                        nc.scalar.activation(
                            HT[:, half * 4 : half * 4 + 4, tt * 128 : (tt + 1) * 128],
                            ptr,
                            AF.Copy,
                        )

            def mm_lhsw_to_ct(dst, wtiles, cts, act=None):
                """dst[:, i, t] = wtiles[ct].T @ HT for i, ct in enumerate(cts).
                wtiles[ct][eo] is a contiguous [128,128] DRAM tile.
                act=None -> DVE copy evac; else ScalarE activation evac."""
                for i, ct in enumerate(cts):
                    wts = []
                    for eo in range(NEO):
                        wt = wl.tile([128, 128], bf16, tag="w_l", name=f"wl{ct}_{eo}")
                        nc.sync.dma_start(wt, wtiles[ct, eo])
                        wts.append(wt)
                    # both 512-chunks per eo so consecutive matmuls share the
                    # stationary operand (halves LDWEIGHTS traffic)
                    pts = [
                        ps.tile([128, 512], f32, tag="mm", name=f"p{ct}_{ch}")
                        for ch in range(2)
                    ]
                    for eo in range(NEO):
                        for ch in range(2):
                            nc.tensor.matmul(
                                pts[ch],
                                wts[eo],
                                HT[:, eo, ch * 512 : (ch + 1) * 512],
                                start=(eo == 0),
                                stop=(eo == NEO - 1),
                                skip_group_check=True,
                            )
                    for ch in range(2):
                        d = dst[:, i, ch * 512 : (ch + 1) * 512]
                        if act is None:
                            nc.vector.tensor_copy(d, pts[ch])
                        else:
                            nc.scalar.activation(d, pts[ch], act)

            def mm_rhs_phase(lhsT3, rtiles, out_fn, n_k, tts=range(NTT)):
                """out[tt] = sum_k lhsT3[:, k, tt*128:+128].T @ rtiles[k].
                rtiles[k] is a contiguous [128,512] DRAM tile."""
                wts = []
                for k in range(n_k):
                    wt = wr.tile([128, 512], bf16, tag="w_r", name=f"wr{k}")
                    nc.sync.dma_start(wt, rtiles[k])
                    wts.append(wt)
                for tt in tts:
                    pt = ps.tile([128, 512], f32, tag="mm", name=f"pv{tt}")
                    for k in range(n_k):
                        nc.tensor.matmul(
                            pt,
                            lhsT3[:, k, tt * 128 : (tt + 1) * 128],
                            wts[k],
                            start=(k == 0),
                            stop=(k == n_k - 1),
                        )
                    out_fn(tt, pt)

            for l in range(nl):
                # ===== attention =====
                layernorm_into_HT(X)
                qk = scrA.tile([128, 16, T], bf16, tag="scrA", name=f"qk{l}")
                QT = qk[:, 0:8, :]
                KT = qk[:, 8:16, :]
                vatt = scrB.tile([128, 26880], bf16, tag="scrB", name=f"vatt{l}")
                VG = vatt[:, 0:8320].rearrange("p (tt h c) -> p tt h c", tt=NTT, h=H)
                attbuf = [vatt[:, 8320 + i * 4640 : 8320 + (i + 1) * 4640] for i in range(4)]
                # ones column per head for the fused softmax denominator
                nc.gpsimd.memset(vatt[:, 0:8320].rearrange("p (a c) -> p a c", c=65)[:, :, 64:65], 1.0)

                def emit_q(ct):
                    mm_lhsw_to_ct(QT[:, ct : ct + 1, :], wqk[l], [ct])

                def emit_k(ct):
                    mm_lhsw_to_ct(KT[:, ct : ct + 1, :], wqk[l], [8 + ct])

                def v_out(tt, pt, ch):
                    # psum [128, 512] = heads ch*8..ch*8+7, 64 cols each -> 65-strided VG
                    nc.vector.tensor_copy(
                        VG[:, tt, ch * 8 : (ch + 1) * 8, 0:64],
                        pt.rearrange("p (h c) -> p h c", h=8),
                    )

                def emit_v(ch, tts):
                    mm_rhs_phase(
                        HT,
                        wv[l, ch],
                        lambda tt, pt, ch=ch: v_out(tt, pt, ch),
                        NEO,
                        tts=tts,
                    )

                # upfront: just enough for pair 0 (Q/K ct0, V tt0-3 of ch0);
                # the rest is spread as fillers so every pair has PE work to
                # overlap its exp drain (pair p's Q/K ct arrive a pair early)
                emit_q(0)
                emit_k(0)
                emit_v(0, range(0, 4))

                units = {
                    0: [lambda: emit_q(1), lambda: emit_k(1),
                        lambda: emit_v(0, range(4, NTT))],
                    2: [lambda: emit_q(2), lambda: emit_k(2)],
                    4: [lambda: emit_q(3), lambda: emit_k(3)],
                    6: [lambda: emit_q(4), lambda: emit_k(4),
                        lambda: emit_v(1, range(0, 4))],
                    8: [lambda: emit_v(1, range(4, NTT)),
                        lambda: emit_q(5), lambda: emit_k(5)],
                    10: [lambda: emit_q(6), lambda: emit_k(6)],
                    12: [lambda: emit_q(7), lambda: emit_k(7)],
                }

                def head_qk_pair(h0, h1, at0, at1, fillers=()):
                    """QK^T + exp + causal mask for a head pair. The two heads'
                    matmuls are K=64 row-tiled (rows 0-63 / 64-127) and emitted
                    adjacently per j so they run concurrently on the PE array.
                    fillers: closures emitting independent PE work, interleaved
                    into the j loop so PE stays busy while ScalarE drains exp."""
                    fillers = list(fillers)
                    offs = []
                    col = 0
                    for j in range(NTT):
                        qn = T - j * 128
                        offs.append(col)
                        col += qn
                    for j in range(NTT):
                        if j in (2, 4, 6) and fillers:
                            fillers.pop(0)()
                        qn = T - j * 128
                        c0 = offs[j]
                        for hh, attT in ((h0, at0), (h1, at1)):
                            ct, ro = hh // 2, (hh % 2) * 64
                            qT = QT[ro : ro + 64, ct, :]
                            kT = KT[ro : ro + 64, ct, :]
                            pa = ps.tile([128, 1024], f32, tag="mm", name=f"pa{l}_{hh}_{j}")
                            for ch in range(0, qn, 512):
                                w = min(512, qn - ch)
                                nc.tensor.matmul(
                                    pa[:, ch : ch + w],
                                    kT[:, j * 128 : (j + 1) * 128],
                                    qT[:, j * 128 + ch : j * 128 + ch + w],
                                    start=True,
                                    stop=(ch > 0),
                                    skip_group_check=True,
                                )
                            # additive causal mask on the diagonal 128 cols via PE
                            nc.tensor.matmul(
                                pa[:, 0:128],
                                ident,
                                maskT,
                                start=False,
                                stop=True,
                                skip_group_check=True,
                            )
                            nc.scalar.activation(attT[:, c0 : c0 + qn], pa[:, :qn], AF.Exp)
                    for fn in fillers:  # drain any leftovers
                        fn()
                    return offs

                def head_av(hh, attT, offs):
                    """AV (with fused denominator row) + normalize into AOT."""
                    av_ps = psb.tile([65, 1024], f32, tag="av", name=f"av{l}_{hh}")
                    for j in range(NTT):
                        for ca in range(2):
                            lo = ca * 512
                            if lo + 512 <= j * 128:
                                continue
                            s = max(lo, j * 128)
                            w = lo + 512 - s
                            nc.tensor.matmul(
                                av_ps[:, s : s + w],
                                VG[:, j, hh, 0:65],
                                attT[:, offs[j] + (s - j * 128) : offs[j] + (s - j * 128) + w],
                                start=(j == 0),
                                stop=(j == 4 * ca + 3),
                                skip_group_check=True,
                            )
                    rden = stat.tile([1, 1024], f32, tag="rden", name=f"rd{l}_{hh}")
                    if F_RECIP == "approx":
                        nc.vector.reciprocal_approx_fast(rden, av_ps[64:65, :])
                    elif F_RECIP == "approx_sbuf":
                        dsb = stat.tile([1, 1024], f32, tag="dsb", name=f"ds{l}_{hh}")
                        nc.scalar.activation(dsb, av_ps[64:65, :], AF.Copy)
                        nc.vector.reciprocal_approx_fast(rden, dsb)
                    else:
                        nc.vector.reciprocal(rden, av_ps[64:65, :])
                    rdd = dram.tile([1, 1024], f32, tag="rdd", name=f"rdd{l}_{hh}")
                    # gpsimd queue: keeps the sem-waiting denominator DMAs from
                    # head-of-line-blocking weight prefetches on the sync queue
                    nc.gpsimd.dma_start(rdd, rden)
                    rdb = bc.tile([64, 1024], f32, tag="rdb", name=f"rdb{l}_{hh}")
                    nc.gpsimd.dma_start(rdb, rdd.to_broadcast([64, 1024]))
                    co, ro2 = hh // 2, (hh % 2) * 64
                    nc.vector.tensor_tensor(
                        AOT[ro2 : ro2 + 64, co, :], av_ps[0:64, :], rdb, ALU.mult
                    )

                # head loop: pairs share the PE array via row tiling (K=64)
                for p in range(8):
                    h0, h1 = 2 * p, 2 * p + 1
                    at0 = attbuf[(2 * p) % 4]
                    at1 = attbuf[(2 * p + 1) % 4]
                    # qkv units interleaved into the QK j loop keep PE busy
                    # while ScalarE drains exp
                    offs = head_qk_pair(
                        h0, h1, at0, at1,
                        fillers=units.get(h0, []) + units.get(h1, []),
                    )
                    head_av(h0, at0, offs)
                    head_av(h1, at1, offs)

                # proj + residual
                for ch in range(2):

                    def proj_out(tt, pt, ch=ch):
                        nc.vector.tensor_tensor(
                            X[:, tt, ch * 512 : (ch + 1) * 512],
                            X[:, tt, ch * 512 : (ch + 1) * 512],
                            pt,
                            ALU.add,
                        )

                    mm_rhs_phase(AOT, wproj[l, ch], proj_out, NEO)

                # ===== mlp =====
                layernorm_into_HT(X)
                H1T = scrA.tile([128, 8, T], bf16, tag="scrA", name=f"h1t{l}")
                FC2A = scrB.tile([128, NTT, T], f32, tag="scrB", name=f"fc2a{l}")
                for slab in range(4):  # 4E in 4 slabs of 1024
                    mm_lhsw_to_ct(H1T, wfc[l], range(slab * 8, slab * 8 + 8), act=AF.Gelu_apprx_tanh)
                    for ch in range(2):
                        if slab == 0:

                            def fc2_out(tt, pt, ch=ch):
                                nc.vector.tensor_tensor(
                                    FC2A[:, tt, ch * 512 : (ch + 1) * 512],
                                    X[:, tt, ch * 512 : (ch + 1) * 512],
                                    pt,
                                    ALU.add,
                                )
                        elif slab < 3:

                            def fc2_out(tt, pt, ch=ch):
                                nc.vector.tensor_tensor(
                                    FC2A[:, tt, ch * 512 : (ch + 1) * 512],
                                    FC2A[:, tt, ch * 512 : (ch + 1) * 512],
                                    pt,
                                    ALU.add,
                                )
                        else:

                            def fc2_out(tt, pt, ch=ch):
                                nc.vector.tensor_tensor(
                                    X[:, tt, ch * 512 : (ch + 1) * 512],
                                    FC2A[:, tt, ch * 512 : (ch + 1) * 512],
                                    pt,
                                    ALU.add,
                                )

                        mm_rhs_phase(H1T, wfc2[l, slab, ch], fc2_out, 8)

            # ===== final layernorm on last token tile, emit last row =====
            st = stat.tile([128, 2, 6], f32, tag="bnst", name="stf")
            for c in range(2):
                nc.vector.bn_stats(st[:, c, :], X[:, NTT - 1, c * 512 : (c + 1) * 512])
            mv = stat.tile([128, 2], f32, tag="bnmv", name="mvf")
            nc.vector.bn_aggr(mv, st)
            rstd = stat.tile([128, 1], f32, tag="rstd", name="rsf")
            if F_RSQRT == "fused":
                nc.scalar.activation(rstd, mv[:, 1:2], AF.Abs_reciprocal_sqrt, bias=eps_t)
            else:
                nc.scalar.activation(rstd, mv[:, 1:2], AF.Sqrt, bias=eps_t)
                nc.vector.reciprocal(rstd, rstd)
            xn = hpool.tile([128, T], f32, tag="hf", name="xnf")
            nc.vector.tensor_scalar(
                out=xn,
                in0=X[:, NTT - 1, :],
                scalar1=mv[:, 0:1],
                scalar2=rstd,
                op0=ALU.subtract,
                op1=ALU.mult,
            )
            nc.sync.dma_start(xlast[:, :], xn[127:128, :])

            if F_P2 == "merged":
                # all-gather the 8 cores' last-position vectors, then compute
                # this core's vocab shard of the logits for all 8 sequences
                nc.sync.dma_start(cc_in[:, :], xn[127:128, :])
                nc.gpsimd.collective_compute(
                    "AllGather",
                    ALU.bypass,
                    replica_groups=[list(range(NCORES))],
                    ins=[cc_in[:, :]],
                    outs=[cc_out[:, :]],
                )
                xf = singles.tile([128, NEO, NCORES], f32)
                for eo in range(NEO):
                    nc.sync.dma_start(
                        xf[:, eo, :],
                        cc_out[:, eo * 128 : (eo + 1) * 128].rearrange("s p -> p s"),
                    )
                xt = singles.tile([128, NEO, NCORES], bf16)
                nc.vector.tensor_copy(xt, xf)
                for vc in range(VSP // 512):
                    pt2 = ps.tile([NCORES, 512], f32, tag="mm", name=f"lg{vc}")
                    for eo in range(NEO):
                        wt = wr.tile([128, 512], bf16, tag="w_r", name=f"wv{vc}_{eo}")
                        nc.sync.dma_start(
                            wt, wtet[eo * 128 : (eo + 1) * 128, vc * 512 : (vc + 1) * 512]
                        )
                        nc.tensor.matmul(
                            pt2, xt[:, eo, :], wt, start=(eo == 0), stop=(eo == NEO - 1)
                        )
                    ot = stat.tile([NCORES, 512], f32, tag="lgo", name=f"lo{vc}")
                    nc.scalar.activation(ot, pt2, AF.Copy)
                    nc.sync.dma_start(lg[:, vc * 512 : (vc + 1) * 512], ot)

    nc.compile()
    return nc


def _build_phase2():
    import concourse.mybir as mybir
    import concourse.tile as tile
    from concourse import bacc

    f32 = mybir.dt.float32
    bf16 = mybir.dt.bfloat16
    AF = mybir.ActivationFunctionType

    nc = bacc.Bacc("TRN2", target_bir_lowering=False)
    xallt = nc.dram_tensor("xallt", [E, NCORES], bf16, kind="ExternalInput")
    wtet = nc.dram_tensor("wtet", [E, VSP], bf16, kind="ExternalInput")
    lg = nc.dram_tensor("lg", [NCORES, VSP], f32, kind="ExternalOutput")

    with tile.TileContext(nc) as tc:
        with (
            tc.tile_pool(name="s", bufs=1) as s,
            tc.tile_pool(name="w", bufs=6) as w,
            tc.tile_pool(name="o", bufs=4) as o,
            tc.tile_pool(name="p", bufs=4, space="PSUM") as p,
        ):
            xt = s.tile([128, NEO, NCORES], bf16)
            nc.sync.dma_start(xt, xallt[:, :].rearrange("(eo p) s -> p eo s", p=128))
            for vc in range(VSP // 512):
                pt = p.tile([NCORES, 512], f32, tag="p", name=f"p{vc}")
                for eo in range(NEO):
                    wt = w.tile([128, 512], bf16, tag="w", name=f"w{vc}_{eo}")
                    nc.sync.dma_start(
                        wt, wtet[eo * 128 : (eo + 1) * 128, vc * 512 : (vc + 1) * 512]
                    )
                    nc.tensor.matmul(pt, xt[:, eo, :], wt, start=(eo == 0), stop=(eo == NEO - 1))
                ot = o.tile([NCORES, 512], f32, tag="o", name=f"o{vc}")
                nc.scalar.activation(ot, pt, AF.Copy)
                nc.sync.dma_start(lg[:, vc * 512 : (vc + 1) * 512], ot)
    nc.compile()
    return nc


def _host_prep(idx, wte, wpe, ln1_w, ln1_b, attn_w, attn_b, proj_w, proj_b,
               ln2_w, ln2_b, fc_w, fc_b, fc2_w, fc2_b, lnf_w, lnf_b, nl):
    import ml_dtypes

    f = np.float32
    bf = ml_dtypes.bfloat16
    idx = np.asarray(idx)
    wte = np.asarray(wte, f)
    wpe = np.asarray(wpe, f)
    x0_all = wte[idx] + wpe[None, :T]  # [8, T, E]

    attn_w = np.asarray(attn_w, f)
    ln1_w = np.asarray(ln1_w, f)
    fc_w = np.asarray(fc_w, f)
    ln2_w = np.asarray(ln2_w, f)

    # fold ln scale into following weights; fold 1/sqrt(D) into W_q
    wqkv = attn_w * ln1_w[:, :, None]
    wqkv[:, :, :E] *= 1.0 / np.sqrt(D)
    wfc = fc_w * ln2_w[:, :, None]

    # biases: must be zero (true for this model); free-dim bias adds unsupported
    bqkv = np.einsum("le,lec->lc", np.asarray(ln1_b, f), attn_w) + np.asarray(attn_b, f)
    bfc = np.einsum("le,lec->lc", np.asarray(ln2_b, f), fc_w) + np.asarray(fc_b, f)
    for nm, b in [("bqkv", bqkv), ("proj_b", np.asarray(proj_b, f)),
                  ("bfc", bfc), ("fc2_b", np.asarray(fc2_b, f)),
                  ("lnf_b", np.asarray(lnf_b, f))]:
        assert np.abs(b).max() == 0.0, f"nonzero bias {nm} not supported by this kernel"

    wtet = np.ascontiguousarray((wte * np.asarray(lnf_w, f)[None, :]).T)  # [E, V]
    shards = []
    for c in range(NCORES):
        sl = wtet[:, c * VSH : min(V, (c + 1) * VSH)]
        pad = np.zeros((E, VSP), f)
        pad[:, : sl.shape[1]] = sl
        shards.append(pad.astype(bf))

    # repack weights into contiguous per-tile layouts (see _build_phase1)
    wqkv = wqkv[:nl].astype(bf)
    proj = np.asarray(proj_w, f)[:nl].astype(bf)
    wfc = wfc[:nl].astype(bf)
    fc2 = np.asarray(fc2_w, f)[:nl].astype(bf)
    # wqk: [nl, 16, NEO, 128, 128] (ct 0-7 = Q, 8-15 = K)
    wqk_t = np.ascontiguousarray(
        wqkv[:, :, : 2 * E]
        .reshape(nl, NEO, 128, 16, 128)
        .transpose(0, 3, 1, 2, 4)
    )
    # wv: [nl, 2, NEO, 128, 512]
    wv_t = np.ascontiguousarray(
        wqkv[:, :, 2 * E :]
        .reshape(nl, NEO, 128, 2, 512)
        .transpose(0, 3, 1, 2, 4)
    )
    # wproj: [nl, 2, NEO, 128, 512]
    wproj_t = np.ascontiguousarray(
        proj.reshape(nl, NEO, 128, 2, 512).transpose(0, 3, 1, 2, 4)
    )
    # wfc: [nl, 32, NEO, 128, 128]
    wfc_t = np.ascontiguousarray(
        wfc.reshape(nl, NEO, 128, 32, 128).transpose(0, 3, 1, 2, 4)
    )
    # wfc2: [nl, 4, 2, 8, 128, 512]
    wfc2_t = np.ascontiguousarray(
        fc2.reshape(nl, 4, 8, 128, 2, 512).transpose(0, 1, 4, 2, 3, 5)
    )
    return (
        np.ascontiguousarray(x0_all, f),
        wqk_t,
        wv_t,
        wproj_t,
        wfc_t,
        wfc2_t,
        shards,
    )


def kernel(idx, wte, wpe, ln1_w, ln1_b, attn_w, attn_b, proj_w, proj_b,
           ln2_w, ln2_b, fc_w, fc_b, fc2_w, fc2_b, lnf_w, lnf_b):
    from concourse.bass_utils import run_bass_kernel_spmd

    x0_all, wqk, wv, wproj, wfc, wfc2, shards = _host_prep(
        idx, wte, wpe, ln1_w, ln1_b, attn_w, attn_b, proj_w, proj_b,
        ln2_w, ln2_b, fc_w, fc_b, fc2_w, fc2_b, lnf_w, lnf_b, NL)

    if "p1" not in _CACHE:
        _CACHE["p1"] = _build_phase1(NL)
    nc1 = _CACHE["p1"]
    in_maps = [
        {"x0": x0_all[c], "wqk": wqk, "wv": wv, "wproj": wproj, "wfc": wfc, "wfc2": wfc2}
        for c in range(NCORES)
    ]
    if F_P2 == "merged":
        for c in range(NCORES):
            in_maps[c]["wtet"] = shards[c]
    trace = os.environ.get("GPT_TRACE", "0") == "1"
    r1 = run_bass_kernel_spmd(nc1, in_maps, core_ids=list(range(NCORES)), trace=trace)
    _CACHE["r1"] = r1

    logits = np.zeros((NCORES, 1, V), np.float32)
    if F_P2 == "merged":
        for c in range(NCORES):
            w = min(V, (c + 1) * VSH) - c * VSH
            logits[:, 0, c * VSH : c * VSH + w] = r1.results[c]["lg"][:, :w]
        return logits

    import ml_dtypes

    xall = np.stack([r1.results[c]["xlast"][0] for c in range(NCORES)])  # [8, E]
    xallt = np.ascontiguousarray(xall.T).astype(ml_dtypes.bfloat16)  # [E, 8]

    if "p2" not in _CACHE:
        _CACHE["p2"] = _build_phase2()
    nc2 = _CACHE["p2"]
    in_maps2 = [{"xallt": xallt, "wtet": shards[c]} for c in range(NCORES)]
    r2 = run_bass_kernel_spmd(nc2, in_maps2, core_ids=list(range(NCORES)), trace=trace)
    _CACHE["r2"] = r2

    for c in range(NCORES):
        w = min(V, (c + 1) * VSH) - c * VSH
        logits[:, 0, c * VSH : c * VSH + w] = r2.results[c]["lg"][:, :w]
    return logits


# revision 64
# speedup vs baseline: 1.1113x; 1.0034x over previous
"""GPT-2 (L=12, E=1024, H=16, T=1024, B=8) forward on 8 Trainium2 NeuronCores.

Strategy: data-parallel over batch (1 sequence per core) for the 12 transformer
layers; vocab-parallel lm_head (each core computes a V/8 logits shard for all
8 sequences) as a second small NEFF, with the 8 last-position vectors gathered
on host between phases. (GPT_P2=merged fuses the lm_head into the first NEFF
via a device AllGather; measured slower, so off by default.)

Host-side preprocessing (all linear folds, no model compute):
  - embedding gather x0 = wte[idx] + wpe  (pure indexing)
  - layernorm scale folded into the following matmul weights
  - 1/sqrt(D) folded into W_q
  - wte transposed (+ lnf scale) for the lm_head
  - all layer weights cast to bf16 and repacked into contiguous per-tile
    blocks so every weight DMA is a sequential read

Device layouts per core (P = 128 partitions):
  - residual X: [t=128, tt=8, e=1024] f32 (token-major)
  - LN output transposed HT: [e=128, eo=8, t=1024] bf16 via PE transposes
  - q,k transposed [c, t] bf16; v natural [t, c] bf16 with a ones column
    appended per head (65-wide lhsT) so the AV matmul also produces the
    softmax denominator as psum row 64 (no separate denominator matmuls)
  - attention transposed: attT[k, q] = (kT-slice).T @ qT into [128,1024]
    2-bank psums; additive -30 causal mask accumulated onto the diag block
    by a PE matmul (ident.T @ maskT); exp on ScalarE psum->sbuf (bf16);
    per-q normalization at AV-output evacuation via
    reciprocal_approx_fast (SBUF-staged) + broadcast DMA
  - QK matmuls for head pairs are row-tiled (K=64, rows 0-63 / 64-127) and
    emitted adjacently per j-block so both run concurrently on the PE array
  - qkv matmul units for later head pairs are interleaved into the QK j-loop
    so PE stays busy (and HAM stays warm) while ScalarE drains exp
  - denominator DMAs ride the gpsimd queue so they never head-of-line-block
    weight prefetches on the sync queue
"""

import os
import sys

import numpy as np

sys.path.insert(0, "/opt/trn_rl_repo")

V, BLK, L, H, E = 50257, 1024, 12, 16, 1024
T = 1024
D = E // H  # 64
NCORES = 8
E3 = 3 * E
E4 = 4 * E
NTT = T // 128  # 8 token tiles
NEO = E // 128  # 8 embed tiles
VSH = (V + NCORES - 1) // NCORES  # 6283 vocab shard
VSP = 13 * 512  # 6656 padded shard width
NL = int(os.environ.get("GPT_NL", str(L)))
# debug flags for isolating numerics regressions
F_RSQRT = os.environ.get("GPT_RSQRT", "fused")  # fused | split
F_RECIP = os.environ.get("GPT_RECIP", "approx_sbuf")  # exact | approx | approx_sbuf
F_P2 = os.environ.get("GPT_P2", "split")  # split | merged

_CACHE = {}


def _build_phase1(nl):
    import concourse.mybir as mybir
    import concourse.tile as tile
    from concourse import bacc
    from concourse.masks import make_identity

    f32 = mybir.dt.float32
    bf16 = mybir.dt.bfloat16
    AF = mybir.ActivationFunctionType
    ALU = mybir.AluOpType

    nc = bacc.Bacc("TRN2", target_bir_lowering=False, num_devices=NCORES)

    x0 = nc.dram_tensor("x0", [T, E], f32, kind="ExternalInput")
    # weights repacked host-side into contiguous per-tile layouts
    wqk = nc.dram_tensor("wqk", [nl, 16, NEO, 128, 128], bf16, kind="ExternalInput")
    wv = nc.dram_tensor("wv", [nl, 2, NEO, 128, 512], bf16, kind="ExternalInput")
    wproj = nc.dram_tensor("wproj", [nl, 2, NEO, 128, 512], bf16, kind="ExternalInput")
    wfc = nc.dram_tensor("wfc", [nl, 32, NEO, 128, 128], bf16, kind="ExternalInput")
    wfc2 = nc.dram_tensor("wfc2", [nl, 4, 2, 8, 128, 512], bf16, kind="ExternalInput")
    xlast = nc.dram_tensor("xlast", [1, E], f32, kind="ExternalOutput")
    if F_P2 == "merged":
        wtet = nc.dram_tensor("wtet", [E, VSP], bf16, kind="ExternalInput")
        lg = nc.dram_tensor("lg", [NCORES, VSP], f32, kind="ExternalOutput")
        cc_in = nc.dram_tensor("cc_in", [1, E], f32, kind="Internal")
        cc_out = nc.dram_tensor(
            "cc_out", [NCORES, E], f32, kind="Internal", addr_space="Shared"
        )

    with tile.TileContext(nc) as tc:
        import contextlib

        ctx = contextlib.ExitStack()
        with ctx:
            singles = ctx.enter_context(tc.tile_pool(name="singles", bufs=1))
            wl = ctx.enter_context(tc.tile_pool(name="wl", bufs=16))  # lhsT [128,128]
            wr = ctx.enter_context(tc.tile_pool(name="wr", bufs=12))  # rhs [128,512]
            hpool = ctx.enter_context(tc.tile_pool(name="hpool", bufs=3))
            stat = ctx.enter_context(tc.tile_pool(name="stat", bufs=2))
            bc = ctx.enter_context(tc.tile_pool(name="bc", bufs=2))
            dram = ctx.enter_context(tc.tile_pool(name="dram", bufs=2, space="DRAM"))
            ps = ctx.enter_context(tc.tile_pool(name="ps", bufs=2, space="PSUM"))
            psb = ctx.enter_context(tc.tile_pool(name="psb", bufs=2, space="PSUM"))

            # ---- persistent tiles ----
            X = singles.tile([128, NTT, T], f32)  # residual [t, tt, e]
            HT = singles.tile([128, NEO, T], bf16)  # ln-out transposed [e, eo, t]
            AOT = singles.tile([128, NEO, T], bf16)  # attn outT [c, co, t]
            # time-shared scratch: attn QT/KT vs mlp H1T; attn V+attT vs mlp FC2A
            scrA = ctx.enter_context(tc.tile_pool(name="scrA", bufs=1))
            scrB = ctx.enter_context(tc.tile_pool(name="scrB", bufs=1))

            ident = singles.tile([128, 128], bf16)
            make_identity(nc, ident)
            eps_t = singles.tile([128, 1], f32)
            nc.gpsimd.memset(eps_t, 1e-5)
            # maskT[k, q] = 0 if q >= k else -30 (additive transposed causal
            # mask); added to a QK psum diag block via ident.T @ maskT = maskT
            maskT = singles.tile([128, 128], bf16)
            nc.gpsimd.memset(maskT, 0.0)
            nc.gpsimd.affine_select(
                out=maskT,
                in_=maskT,
                compare_op=ALU.is_ge,
                fill=-30.0,
                base=0,
                pattern=[[1, 128]],
                channel_multiplier=-1,
            )

            # ---- load x0 ----
            x0v = x0[:, :].rearrange("(tt p) e -> p tt e", p=128)
            for tt in range(NTT):
                nc.sync.dma_start(X[:, tt, :], x0v[:, tt, :])

            def emit_stats(src, tt):
                """bn stats + rsqrt for one token tile of src (DVE+ACT)."""
                st = stat.tile([128, 2, 6], f32, tag="bnst", name=f"st{tt}")
                for c in range(2):
                    nc.vector.bn_stats(st[:, c, :], src[:, tt, c * 512 : (c + 1) * 512])
                mv = stat.tile([128, 2], f32, tag="bnmv", name=f"mv{tt}", bufs=10)
                nc.vector.bn_aggr(mv, st)
                rstd = stat.tile([128, 1], f32, tag="rstd", name=f"rs{tt}", bufs=10)
                if F_RSQRT == "fused":
                    nc.scalar.activation(rstd, mv[:, 1:2], AF.Abs_reciprocal_sqrt, bias=eps_t)
                else:
                    nc.scalar.activation(rstd, mv[:, 1:2], AF.Sqrt, bias=eps_t)
                    nc.vector.reciprocal(rstd, rstd)
                return mv, rstd

            # stats emitted early (inside proj/fc2 evac callbacks) carry here
            carry_stats = []

            def layernorm_into_HT(src):
                """LN(src[t,tt,e]) -> HT[e,eo,t] (transposed, bf16) via PE.
                Uses carry_stats if the producer phase already emitted them."""
                pre = carry_stats[:NTT] if len(carry_stats) == NTT else None
                del carry_stats[:]
                for tt in range(NTT):
                    mv, rstd = pre[tt] if pre else emit_stats(src, tt)
                    h = hpool.tile([128, T], bf16, tag="h", name=f"h{tt}")
                    nc.vector.tensor_scalar(
                        out=h,
                        in0=src[:, tt, :],
                        scalar1=mv[:, 0:1],
                        scalar2=rstd,
                        op0=ALU.subtract,
                        op1=ALU.mult,
                    )
                    for half in range(2):
                        ptr = ps.tile([128, 4, 128], bf16, tag="mm", name=f"ptr{tt}_{half}")
                        for eq in range(4):
                            eo = half * 4 + eq
                            nc.tensor.transpose(ptr[:, eq, :], h[:, eo * 128 : (eo + 1) * 128], ident)
                        nc.scalar.activation(
                            HT[:, half * 4 : half * 4 + 4, tt * 128 : (tt + 1) * 128],
                            ptr,
                            AF.Copy,
                        )

            def mm_lhsw_to_ct(dst, wtiles, cts, act=None):
                """dst[:, i, t] = wtiles[ct].T @ HT for i, ct in enumerate(cts).
                wtiles[ct][eo] is a contiguous [128,128] DRAM tile.
                act=None -> DVE copy evac; else ScalarE activation evac."""
                for i, ct in enumerate(cts):
                    wts = []
                    for eo in range(NEO):
                        wt = wl.tile([128, 128], bf16, tag="w_l", name=f"wl{ct}_{eo}")
                        nc.sync.dma_start(wt, wtiles[ct, eo])
                        wts.append(wt)
                    # both 512-chunks per eo so consecutive matmuls share the
                    # stationary operand (halves LDWEIGHTS traffic)
                    pts = [
                        ps.tile([128, 512], f32, tag="mm", name=f"p{ct}_{ch}")
                        for ch in range(2)
                    ]
                    for eo in range(NEO):
                        for ch in range(2):
                            nc.tensor.matmul(
                                pts[ch],
                                wts[eo],
                                HT[:, eo, ch * 512 : (ch + 1) * 512],
                                start=(eo == 0),
                                stop=(eo == NEO - 1),
                                skip_group_check=True,
                            )
                    for ch in range(2):
                        d = dst[:, i, ch * 512 : (ch + 1) * 512]
                        if act is None:
                            nc.vector.tensor_copy(d, pts[ch])
                        else:
                            nc.scalar.activation(d, pts[ch], act)

            def mm_rhs_phase(lhsT3, rtiles, out_fn, n_k, tts=range(NTT)):
                """out[tt] = sum_k lhsT3[:, k, tt*128:+128].T @ rtiles[k].
                rtiles[k] is a contiguous [128,512] DRAM tile."""
                wts = []
                for k in range(n_k):
                    wt = wr.tile([128, 512], bf16, tag="w_r", name=f"wr{k}")
                    nc.sync.dma_start(wt, rtiles[k])
                    wts.append(wt)
                for tt in tts:
                    pt = ps.tile([128, 512], f32, tag="mm", name=f"pv{tt}")
                    for k in range(n_k):
                        nc.tensor.matmul(
                            pt,
                            lhsT3[:, k, tt * 128 : (tt + 1) * 128],
                            wts[k],
                            start=(k == 0),
                            stop=(k == n_k - 1),
                        )
                    out_fn(tt, pt)

            for l in range(nl):
                # ===== attention =====
                layernorm_into_HT(X)
                qk = scrA.tile([128, 16, T], bf16, tag="scrA", name=f"qk{l}")
                QT = qk[:, 0:8, :]
                KT = qk[:, 8:16, :]
                vatt = scrB.tile([128, 26880], bf16, tag="scrB", name=f"vatt{l}")
                VG = vatt[:, 0:8320].rearrange("p (tt h c) -> p tt h c", tt=NTT, h=H)
                attbuf = [vatt[:, 8320 + i * 4640 : 8320 + (i + 1) * 4640] for i in range(4)]
                # ones column per head for the fused softmax denominator
                nc.gpsimd.memset(vatt[:, 0:8320].rearrange("p (a c) -> p a c", c=65)[:, :, 64:65], 1.0)

                def emit_q(ct):
                    mm_lhsw_to_ct(QT[:, ct : ct + 1, :], wqk[l], [ct])

                def emit_k(ct):
                    mm_lhsw_to_ct(KT[:, ct : ct + 1, :], wqk[l], [8 + ct])

                def v_out(tt, pt, ch):
                    # psum [128, 512] = heads ch*8..ch*8+7, 64 cols each -> 65-strided VG
                    nc.vector.tensor_copy(
                        VG[:, tt, ch * 8 : (ch + 1) * 8, 0:64],
                        pt.rearrange("p (h c) -> p h c", h=8),
                    )

                def emit_v(ch, tts):
                    mm_rhs_phase(
                        HT,
                        wv[l, ch],
                        lambda tt, pt, ch=ch: v_out(tt, pt, ch),
                        NEO,
                        tts=tts,
                    )

                # upfront: just enough for pair 0 (Q/K ct0, V tt0-3 of ch0);
                # the rest is spread as fillers so every pair has PE work to
                # overlap its exp drain (pair p's Q/K ct arrive a pair early)
                emit_q(0)
                emit_k(0)
                emit_v(0, range(0, 4))

                units = {
                    0: [lambda: emit_q(1), lambda: emit_k(1),
                        lambda: emit_v(0, range(4, NTT))],
                    2: [lambda: emit_q(2), lambda: emit_k(2)],
                    4: [lambda: emit_q(3), lambda: emit_k(3)],
                    6: [lambda: emit_q(4), lambda: emit_k(4),
                        lambda: emit_v(1, range(0, 4))],
                    8: [lambda: emit_v(1, range(4, NTT)),
                        lambda: emit_q(5), lambda: emit_k(5)],
                    10: [lambda: emit_q(6), lambda: emit_k(6)],
                    12: [lambda: emit_q(7), lambda: emit_k(7)],
                }

                def head_qk_pair(h0, h1, at0, at1, fillers=()):
                    """QK^T + exp + causal mask for a head pair. The two heads'
                    matmuls are K=64 row-tiled (rows 0-63 / 64-127) and emitted
                    adjacently per j so they run concurrently on the PE array.
                    fillers: closures emitting independent PE work, interleaved
                    into the j loop so PE stays busy while ScalarE drains exp."""
                    fillers = list(fillers)
                    offs = []
                    col = 0
                    for j in range(NTT):
                        qn = T - j * 128
                        offs.append(col)
                        col += qn
                    for j in range(NTT):
                        if j in (2, 4, 6) and fillers:
                            fillers.pop(0)()
                        qn = T - j * 128
                        c0 = offs[j]
                        for hh, attT in ((h0, at0), (h1, at1)):
                            ct, ro = hh // 2, (hh % 2) * 64
                            qT = QT[ro : ro + 64, ct, :]
                            kT = KT[ro : ro + 64, ct, :]
                            pa = ps.tile([128, 1024], f32, tag="mm", name=f"pa{l}_{hh}_{j}")
                            for ch in range(0, qn, 512):
                                w = min(512, qn - ch)
                                nc.tensor.matmul(
                                    pa[:, ch : ch + w],
                                    kT[:, j * 128 : (j + 1) * 128],
                                    qT[:, j * 128 + ch : j * 128 + ch + w],
                                    start=True,
                                    stop=(ch > 0),
                                    skip_group_check=True,
                                )
                            # additive causal mask on the diagonal 128 cols via PE
                            nc.tensor.matmul(
                                pa[:, 0:128],
                                ident,
                                maskT,
                                start=False,
                                stop=True,
                                skip_group_check=True,
                            )
                            nc.scalar.activation(attT[:, c0 : c0 + qn], pa[:, :qn], AF.Exp)
                    for fn in fillers:  # drain any leftovers
                        fn()
                    return offs

                def head_av(hh, attT, offs):
                    """AV (with fused denominator row) + normalize into AOT."""
                    av_ps = psb.tile([65, 1024], f32, tag="av", name=f"av{l}_{hh}")
                    for j in range(NTT):
                        for ca in range(2):
                            lo = ca * 512
                            if lo + 512 <= j * 128:
                                continue
                            s = max(lo, j * 128)
                            w = lo + 512 - s
                            nc.tensor.matmul(
                                av_ps[:, s : s + w],
                                VG[:, j, hh, 0:65],
                                attT[:, offs[j] + (s - j * 128) : offs[j] + (s - j * 128) + w],
                                start=(j == 0),
                                stop=(j == 4 * ca + 3),
                                skip_group_check=True,
                            )
                    rden = stat.tile([1, 1024], f32, tag="rden", name=f"rd{l}_{hh}")
                    if F_RECIP == "approx":
                        nc.vector.reciprocal_approx_fast(rden, av_ps[64:65, :])
                    elif F_RECIP == "approx_sbuf":
                        dsb = stat.tile([1, 1024], f32, tag="dsb", name=f"ds{l}_{hh}")
                        nc.scalar.activation(dsb, av_ps[64:65, :], AF.Copy)
                        nc.vector.reciprocal_approx_fast(rden, dsb)
                    else:
                        nc.vector.reciprocal(rden, av_ps[64:65, :])
                    rdd = dram.tile([1, 1024], f32, tag="rdd", name=f"rdd{l}_{hh}")
                    # gpsimd queue: keeps the sem-waiting denominator DMAs from
                    # head-of-line-blocking weight prefetches on the sync queue
                    nc.gpsimd.dma_start(rdd, rden)
                    rdb = bc.tile([64, 1024], f32, tag="rdb", name=f"rdb{l}_{hh}")
                    nc.gpsimd.dma_start(rdb, rdd.to_broadcast([64, 1024]))
                    co, ro2 = hh // 2, (hh % 2) * 64
                    nc.vector.tensor_tensor(
                        AOT[ro2 : ro2 + 64, co, :], av_ps[0:64, :], rdb, ALU.mult
                    )

                # head loop: pairs share the PE array via row tiling (K=64)
                for p in range(8):
                    h0, h1 = 2 * p, 2 * p + 1
                    at0 = attbuf[(2 * p) % 4]
                    at1 = attbuf[(2 * p + 1) % 4]
                    # qkv units interleaved into the QK j loop keep PE busy
                    # while ScalarE drains exp
                    offs = head_qk_pair(
                        h0, h1, at0, at1,
                        fillers=units.get(h0, []) + units.get(h1, []),
                    )
                    head_av(h0, at0, offs)
                    head_av(h1, at1, offs)

                # proj + residual
                for ch in range(2):

                    def proj_out(tt, pt, ch=ch):
                        nc.vector.tensor_tensor(
                            X[:, tt, ch * 512 : (ch + 1) * 512],
                            X[:, tt, ch * 512 : (ch + 1) * 512],
                            pt,
                            ALU.add,
                        )
                        if ch == 1:
                            carry_stats.append(emit_stats(X, tt))

                    mm_rhs_phase(AOT, wproj[l, ch], proj_out, NEO)

                # ===== mlp =====
                layernorm_into_HT(X)
                H1T = scrA.tile([128, 8, T], bf16, tag="scrA", name=f"h1t{l}")
                FC2A = scrB.tile([128, NTT, T], f32, tag="scrB", name=f"fc2a{l}")
                for slab in range(4):  # 4E in 4 slabs of 1024
                    mm_lhsw_to_ct(H1T, wfc[l], range(slab * 8, slab * 8 + 8), act=AF.Gelu_apprx_tanh)
                    for ch in range(2):
                        if slab == 0:

                            def fc2_out(tt, pt, ch=ch):
                                nc.vector.tensor_tensor(
                                    FC2A[:, tt, ch * 512 : (ch + 1) * 512],
                                    X[:, tt, ch * 512 : (ch + 1) * 512],
                                    pt,
                                    ALU.add,
                                )
                        elif slab < 3:

                            def fc2_out(tt, pt, ch=ch):
                                nc.vector.tensor_tensor(
                                    FC2A[:, tt, ch * 512 : (ch + 1) * 512],
                                    FC2A[:, tt, ch * 512 : (ch + 1) * 512],
                                    pt,
                                    ALU.add,
                                )
                        else:

                            def fc2_out(tt, pt, ch=ch):
                                nc.vector.tensor_tensor(
                                    X[:, tt, ch * 512 : (ch + 1) * 512],
                                    FC2A[:, tt, ch * 512 : (ch + 1) * 512],
                                    pt,
                                    ALU.add,
                                )
                                if ch == 1 and l < nl - 1:
                                    carry_stats.append(emit_stats(X, tt))

                        mm_rhs_phase(H1T, wfc2[l, slab, ch], fc2_out, 8)

            # ===== final layernorm on last token tile, emit last row =====
            st = stat.tile([128, 2, 6], f32, tag="bnst", name="stf")
            for c in range(2):
                nc.vector.bn_stats(st[:, c, :], X[:, NTT - 1, c * 512 : (c + 1) * 512])
            mv = stat.tile([128, 2], f32, tag="bnmv", name="mvf", bufs=10)
            nc.vector.bn_aggr(mv, st)
            rstd = stat.tile([128, 1], f32, tag="rstd", name="rsf", bufs=10)
            if F_RSQRT == "fused":
                nc.scalar.activation(rstd, mv[:, 1:2], AF.Abs_reciprocal_sqrt, bias=eps_t)
            else:
                nc.scalar.activation(rstd, mv[:, 1:2], AF.Sqrt, bias=eps_t)
                nc.vector.reciprocal(rstd, rstd)
            xn = hpool.tile([128, T], f32, tag="hf", name="xnf")
            nc.vector.tensor_scalar(
                out=xn,
                in0=X[:, NTT - 1, :],
                scalar1=mv[:, 0:1],
                scalar2=rstd,
                op0=ALU.subtract,
                op1=ALU.mult,
            )
            nc.sync.dma_start(xlast[:, :], xn[127:128, :])

            if F_P2 == "merged":
                # all-gather the 8 cores' last-position vectors, then compute
                # this core's vocab shard of the logits for all 8 sequences
                nc.sync.dma_start(cc_in[:, :], xn[127:128, :])
                nc.gpsimd.collective_compute(
                    "AllGather",
                    ALU.bypass,
                    replica_groups=[list(range(NCORES))],
                    ins=[cc_in[:, :]],
                    outs=[cc_out[:, :]],
                )
                xf = singles.tile([128, NEO, NCORES], f32)
                for eo in range(NEO):
                    nc.sync.dma_start(
                        xf[:, eo, :],
                        cc_out[:, eo * 128 : (eo + 1) * 128].rearrange("s p -> p s"),
                    )
                xt = singles.tile([128, NEO, NCORES], bf16)
                nc.vector.tensor_copy(xt, xf)
                for vc in range(VSP // 512):
                    pt2 = ps.tile([NCORES, 512], f32, tag="mm", name=f"lg{vc}")
                    for eo in range(NEO):
                        wt = wr.tile([128, 512], bf16, tag="w_r", name=f"wv{vc}_{eo}")
                        nc.sync.dma_start(
                            wt, wtet[eo * 128 : (eo + 1) * 128, vc * 512 : (vc + 1) * 512]
                        )
                        nc.tensor.matmul(
                            pt2, xt[:, eo, :], wt, start=(eo == 0), stop=(eo == NEO - 1)
                        )
                    ot = stat.tile([NCORES, 512], f32, tag="lgo", name=f"lo{vc}")
                    nc.scalar.activation(ot, pt2, AF.Copy)
                    nc.sync.dma_start(lg[:, vc * 512 : (vc + 1) * 512], ot)

    nc.compile()
    return nc


def _build_phase2():
    import concourse.mybir as mybir
    import concourse.tile as tile
    from concourse import bacc

    f32 = mybir.dt.float32
    bf16 = mybir.dt.bfloat16
    AF = mybir.ActivationFunctionType

    nc = bacc.Bacc("TRN2", target_bir_lowering=False)
    xallt = nc.dram_tensor("xallt", [E, NCORES], bf16, kind="ExternalInput")
    wtet = nc.dram_tensor("wtet", [E, VSP], bf16, kind="ExternalInput")
    lg = nc.dram_tensor("lg", [NCORES, VSP], f32, kind="ExternalOutput")

    with tile.TileContext(nc) as tc:
        with (
            tc.tile_pool(name="s", bufs=1) as s,
            tc.tile_pool(name="w", bufs=6) as w,
            tc.tile_pool(name="o", bufs=4) as o,
            tc.tile_pool(name="p", bufs=4, space="PSUM") as p,
        ):
            xt = s.tile([128, NEO, NCORES], bf16)
            nc.sync.dma_start(xt, xallt[:, :].rearrange("(eo p) s -> p eo s", p=128))
            for vc in range(VSP // 512):
                pt = p.tile([NCORES, 512], f32, tag="p", name=f"p{vc}")
                for eo in range(NEO):
                    wt = w.tile([128, 512], bf16, tag="w", name=f"w{vc}_{eo}")
                    nc.sync.dma_start(
                        wt, wtet[eo * 128 : (eo + 1) * 128, vc * 512 : (vc + 1) * 512]
                    )
                    nc.tensor.matmul(pt, xt[:, eo, :], wt, start=(eo == 0), stop=(eo == NEO - 1))
                ot = o.tile([NCORES, 512], f32, tag="o", name=f"o{vc}")
                nc.scalar.activation(ot, pt, AF.Copy)
                nc.sync.dma_start(lg[:, vc * 512 : (vc + 1) * 512], ot)
    nc.compile()
    return nc


def _host_prep(idx, wte, wpe, ln1_w, ln1_b, attn_w, attn_b, proj_w, proj_b,
               ln2_w, ln2_b, fc_w, fc_b, fc2_w, fc2_b, lnf_w, lnf_b, nl):
    import ml_dtypes

    f = np.float32
    bf = ml_dtypes.bfloat16
    idx = np.asarray(idx)
    wte = np.asarray(wte, f)
    wpe = np.asarray(wpe, f)
    x0_all = wte[idx] + wpe[None, :T]  # [8, T, E]

    attn_w = np.asarray(attn_w, f)
    ln1_w = np.asarray(ln1_w, f)
    fc_w = np.asarray(fc_w, f)
    ln2_w = np.asarray(ln2_w, f)

    # fold ln scale into following weights; fold 1/sqrt(D) into W_q
    wqkv = attn_w * ln1_w[:, :, None]
    wqkv[:, :, :E] *= 1.0 / np.sqrt(D)
    wfc = fc_w * ln2_w[:, :, None]

    # biases: must be zero (true for this model); free-dim bias adds unsupported
    bqkv = np.einsum("le,lec->lc", np.asarray(ln1_b, f), attn_w) + np.asarray(attn_b, f)
    bfc = np.einsum("le,lec->lc", np.asarray(ln2_b, f), fc_w) + np.asarray(fc_b, f)
    for nm, b in [("bqkv", bqkv), ("proj_b", np.asarray(proj_b, f)),
                  ("bfc", bfc), ("fc2_b", np.asarray(fc2_b, f)),
                  ("lnf_b", np.asarray(lnf_b, f))]:
        assert np.abs(b).max() == 0.0, f"nonzero bias {nm} not supported by this kernel"

    wtet = np.ascontiguousarray((wte * np.asarray(lnf_w, f)[None, :]).T)  # [E, V]
    shards = []
    for c in range(NCORES):
        sl = wtet[:, c * VSH : min(V, (c + 1) * VSH)]
        pad = np.zeros((E, VSP), f)
        pad[:, : sl.shape[1]] = sl
        shards.append(pad.astype(bf))

    # repack weights into contiguous per-tile layouts (see _build_phase1)
    wqkv = wqkv[:nl].astype(bf)
    proj = np.asarray(proj_w, f)[:nl].astype(bf)
    wfc = wfc[:nl].astype(bf)
    fc2 = np.asarray(fc2_w, f)[:nl].astype(bf)
    # wqk: [nl, 16, NEO, 128, 128] (ct 0-7 = Q, 8-15 = K)
    wqk_t = np.ascontiguousarray(
        wqkv[:, :, : 2 * E]
        .reshape(nl, NEO, 128, 16, 128)
        .transpose(0, 3, 1, 2, 4)
    )
    # wv: [nl, 2, NEO, 128, 512]
    wv_t = np.ascontiguousarray(
        wqkv[:, :, 2 * E :]
        .reshape(nl, NEO, 128, 2, 512)
        .transpose(0, 3, 1, 2, 4)
    )
    # wproj: [nl, 2, NEO, 128, 512]
    wproj_t = np.ascontiguousarray(
        proj.reshape(nl, NEO, 128, 2, 512).transpose(0, 3, 1, 2, 4)
    )
    # wfc: [nl, 32, NEO, 128, 128]
    wfc_t = np.ascontiguousarray(
        wfc.reshape(nl, NEO, 128, 32, 128).transpose(0, 3, 1, 2, 4)
    )
    # wfc2: [nl, 4, 2, 8, 128, 512]
    wfc2_t = np.ascontiguousarray(
        fc2.reshape(nl, 4, 8, 128, 2, 512).transpose(0, 1, 4, 2, 3, 5)
    )
    return (
        np.ascontiguousarray(x0_all, f),
        wqk_t,
        wv_t,
        wproj_t,
        wfc_t,
        wfc2_t,
        shards,
    )


def kernel(idx, wte, wpe, ln1_w, ln1_b, attn_w, attn_b, proj_w, proj_b,
           ln2_w, ln2_b, fc_w, fc_b, fc2_w, fc2_b, lnf_w, lnf_b):
    from concourse.bass_utils import run_bass_kernel_spmd

    x0_all, wqk, wv, wproj, wfc, wfc2, shards = _host_prep(
        idx, wte, wpe, ln1_w, ln1_b, attn_w, attn_b, proj_w, proj_b,
        ln2_w, ln2_b, fc_w, fc_b, fc2_w, fc2_b, lnf_w, lnf_b, NL)

    if "p1" not in _CACHE:
        _CACHE["p1"] = _build_phase1(NL)
    nc1 = _CACHE["p1"]
    in_maps = [
        {"x0": x0_all[c], "wqk": wqk, "wv": wv, "wproj": wproj, "wfc": wfc, "wfc2": wfc2}
        for c in range(NCORES)
    ]
    if F_P2 == "merged":
        for c in range(NCORES):
            in_maps[c]["wtet"] = shards[c]
    trace = os.environ.get("GPT_TRACE", "0") == "1"
    r1 = run_bass_kernel_spmd(nc1, in_maps, core_ids=list(range(NCORES)), trace=trace)
    _CACHE["r1"] = r1

    logits = np.zeros((NCORES, 1, V), np.float32)
    if F_P2 == "merged":
        for c in range(NCORES):
            w = min(V, (c + 1) * VSH) - c * VSH
            logits[:, 0, c * VSH : c * VSH + w] = r1.results[c]["lg"][:, :w]
        return logits

    import ml_dtypes

    xall = np.stack([r1.results[c]["xlast"][0] for c in range(NCORES)])  # [8, E]
    xallt = np.ascontiguousarray(xall.T).astype(ml_dtypes.bfloat16)  # [E, 8]

    if "p2" not in _CACHE:
        _CACHE["p2"] = _build_phase2()
    nc2 = _CACHE["p2"]
    in_maps2 = [{"xallt": xallt, "wtet": shards[c]} for c in range(NCORES)]
    r2 = run_bass_kernel_spmd(nc2, in_maps2, core_ids=list(range(NCORES)), trace=trace)
    _CACHE["r2"] = r2

    for c in range(NCORES):
        w = min(V, (c + 1) * VSH) - c * VSH
        logits[:, 0, c * VSH : c * VSH + w] = r2.results[c]["lg"][:, :w]
    return logits
